# revision 14
# baseline (speedup 1.0000x reference)
"""Trainium2 Bass kernel for nn_Decoder (GRU + Luong attention + greedy decode).

Sharding: hybrid
  - GRU + attention: data-parallel over batch (8 rows/core)
  - fc ([V,H] @ h) + log-softmax stats + argmax: tensor-parallel over vocab
    (6250 rows/core, fc_w slice resident in SBUF)
  - two small AllGathers per step: concat_out ([8,512]->[64,512]) and
    per-shard (max, sumexp, argmax) stats ([128,4]->[1024,4])

Everything on the argmax feedback path is fp32 (top1-top2 logit gaps go down
to 3.5e-5 on these inputs; any bf16 would flip tokens and cascade).
"""

import numpy as np

import concourse.bacc as bacc
import concourse.bass as bass
import concourse.mybir as mybir
import concourse.tile as tile
from concourse.masks import make_identity
from concourse import bass_utils

F32 = mybir.dt.float32
I32 = mybir.dt.int32
U32 = mybir.dt.uint32
AF = mybir.ActivationFunctionType
OP = mybir.AluOpType
AX = mybir.AxisListType

NC = 8          # cores
B = 64          # batch
BC = B // NC    # batch rows per core
S = 64          # source length
E = 256         # embed dim
H = 512         # hidden
V = 50000       # vocab
VC = V // NC    # vocab rows per core
T = 32          # decode steps
SOS = 1

NTILE = 13          # vocab tiles per core: 12 x 512 + 1 x 106
TAIL_W = VC - 12 * 512   # 106
NBANK = 7           # fc psum passes (2 tiles/bank, bank 6 half-used)
STG_W = 6 * 512 + TAIL_W  # staged logits width per partition (3178)
BIG = 1.0e9

LN2 = 0.6931471805599453
# quadratic fit for log2(m), m in [1,2); refined by 2 Newton steps w/ Exp
LC2, LC1, LC0 = -0.344845, 2.024658, -1.674873


def _ap(t, off, dims):
    """Hand-built access pattern view: dims = [[stride, size], ...] (elements)."""
    tensor = t.tensor if isinstance(t, bass.AP) else t
    return bass.AP(tensor, off, dims)


def build(t_steps=T):
    nc = bacc.Bacc("TRN2", target_bir_lowering=False, debug=False, num_devices=NC)

    # ---------------- external inputs (per core) ----------------
    emb_d = nc.dram_tensor("emb", [V, E], F32, kind="ExternalInput")
    x0T_d = nc.dram_tensor("x0T", [128, 2 * BC], F32, kind="ExternalInput")
    h0b_d = nc.dram_tensor("h0b", [BC, H], F32, kind="ExternalInput")
    h0T_d = nc.dram_tensor("h0T", [128, 4 * BC], F32, kind="ExternalInput")
    wih_d = nc.dram_tensor("wihK", [128, 2 * 3 * H], F32, kind="ExternalInput")
    whh_d = nc.dram_tensor("whhK", [128, 4 * 3 * H], F32, kind="ExternalInput")
    WahK_d = nc.dram_tensor("WahK", [128, 4 * H], F32, kind="ExternalInput")
    epT_d = nc.dram_tensor("epT", [128, 4 * BC * S], F32, kind="ExternalInput")
    eoWs_d = nc.dram_tensor("eoWs", [S, BC * H], F32, kind="ExternalInput")
    fcT_d = nc.dram_tensor("fcT", [128, 4 * VC], F32, kind="ExternalInput")
    sel_d = nc.dram_tensor("sel", [B, BC], F32, kind="ExternalInput")
    eye_d = nc.dram_tensor("eye8r", [128, BC * BC], F32, kind="ExternalInput")
    bofs_d = nc.dram_tensor("bankofs", [128, NBANK * 8], F32, kind="ExternalInput")

    # ---------------- external outputs (per core) ----------------
    olog_d = nc.dram_tensor("o_logits", [B, t_steps, VC], F32, kind="ExternalOutput")
    oh_d = nc.dram_tensor("o_h", [BC, H], F32, kind="ExternalOutput")
    otok_d = nc.dram_tensor("o_tok", [B, t_steps], F32, kind="ExternalOutput")

    # ---------------- internal dram (collective bounce) ----------------
    cc1_in = nc.dram_tensor("cc1_in", [BC, H], F32)
    cc1_out = nc.dram_tensor("cc1_out", [B, H], F32, addr_space="Shared")
    cc2_in = nc.dram_tensor("cc2_in", [128, 4], F32)
    cc2_out = nc.dram_tensor("cc2_out", [128 * NC, 4], F32, addr_space="Shared")
    rgroups = [list(range(NC))]

    with tile.TileContext(nc) as tc:
        with tc.tile_pool(name="res", bufs=1) as res, \
             tc.tile_pool(name="state", bufs=1) as stp, \
             tc.tile_pool(name="work", bufs=1) as wk, \
             tc.tile_pool(name="stats", bufs=1) as st, \
             tc.tile_pool(name="stg", bufs=1) as stg, \
             tc.tile_pool(name="psRZ", bufs=1, space="PSUM") as psRZ, \
             tc.tile_pool(name="psCAT", bufs=1, space="PSUM") as psCAT, \
             tc.tile_pool(name="psS", bufs=2, space="PSUM") as psS, \
             tc.tile_pool(name="psDUM", bufs=1, space="PSUM") as psDUM, \
             tc.tile_pool(name="psFC", bufs=2, space="PSUM") as psFC:

            # ---------------- residents ----------------
            ident = res.tile([64, 64], F32)
            make_identity(nc, ident[:, :])
            wih = res.tile([128, 2 * 3 * H], F32)
            nc.sync.dma_start(wih[:, :], wih_d.ap())
            whh = res.tile([128, 4 * 3 * H], F32)
            nc.sync.dma_start(whh[:, :], whh_d.ap())
            WahK = res.tile([128, 4 * H], F32)
            nc.sync.dma_start(WahK[:, :], WahK_d.ap())
            epT = res.tile([128, 4 * BC * S], F32)
            nc.sync.dma_start(epT[:, :], epT_d.ap())
            dumW = res.tile([128, 64], F32)
            nc.gpsimd.memset(dumW[:, :], 0.001)
            eoWs = res.tile([S, BC * H], F32)
            nc.sync.dma_start(eoWs[:, :], eoWs_d.ap())
            fcT = res.tile([128, 4 * VC], F32)
            nc.sync.dma_start(fcT[:, :], fcT_d.ap())
            sel = res.tile([B, BC], F32)
            nc.sync.dma_start(sel[:, :], sel_d.ap())
            eye8 = res.tile([128, BC * BC], F32)
            nc.sync.dma_start(eye8[:, :], eye_d.ap())
            bankofs = res.tile([128, NBANK * 8], F32)
            nc.sync.dma_start(bankofs[:, :], bofs_d.ap())
            toks = res.tile([B, t_steps], F32)

            # initial state
            xT = stp.tile([128, 2 * BC], F32, tag="xT")
            nc.sync.dma_start(xT[:, :], x0T_d.ap())
            h_b = stp.tile([BC, H], F32, tag="h_b")
            nc.sync.dma_start(h_b[:, :], h0b_d.ap())
            hT = stp.tile([128, 4 * BC], F32, tag="hT")
            nc.sync.dma_start(hT[:, :], h0T_d.ap())

            ID8 = ident[0:BC, 0:BC]
            ID64 = ident[0:64, 0:64]

            dum_ps = psDUM.tile([128, 512], F32, space="PSUM", tag="dum")
            nc.tensor.matmul(dum_ps[0:64, :], dumW[:, 0:64], wih[:, 0:512],
                             start=True, stop=False, skip_group_check=True)

            def warm_pe(n_mm):
                for _ in range(n_mm):
                    nc.tensor.matmul(dum_ps[0:64, :], dumW[:, 0:64], wih[:, 0:512],
                                     start=False, stop=False, skip_group_check=True)

            for t in range(t_steps):
                # ============ GRU ============
                rz_ps = psRZ.tile([BC, 1024], F32, space="PSUM", tag="rz")
                in_ps = psS.tile([BC, H], F32, space="PSUM", tag="s")
                hn_ps = psS.tile([BC, H], F32, space="PSUM", tag="s")
                # gh contributions first (only need hT -> overlap the token path)
                for hc in range(4):
                    for nt in range(2):
                        sl = slice(nt * 512, nt * 512 + 512)
                        nc.tensor.matmul(
                            rz_ps[:, sl], hT[:, hc * BC:(hc + 1) * BC],
                            whh[:, hc * 1536 + nt * 512: hc * 1536 + nt * 512 + 512],
                            start=(hc == 0), stop=False)
                    nc.tensor.matmul(
                        hn_ps[:, :], hT[:, hc * BC:(hc + 1) * BC],
                        whh[:, hc * 1536 + 1024: hc * 1536 + 1536],
                        start=(hc == 0), stop=(hc == 3))
                # gi contributions (need xT of this step)
                for ec in range(2):
                    for nt in range(2):
                        sl = slice(nt * 512, nt * 512 + 512)
                        nc.tensor.matmul(
                            rz_ps[:, sl], xT[:, ec * BC:(ec + 1) * BC],
                            wih[:, ec * 1536 + nt * 512: ec * 1536 + nt * 512 + 512],
                            start=False, stop=(ec == 1))
                    nc.tensor.matmul(
                        in_ps[:, :], xT[:, ec * BC:(ec + 1) * BC],
                        wih[:, ec * 1536 + 1024: ec * 1536 + 1536],
                        start=(ec == 0), stop=(ec == 1))

                # gates: sigmoid(x) = 0.5*tanh(0.5x)+0.5 (stay in exp/tanh table set)
                rz = wk.tile([BC, 1024], F32, tag="rz_sb")
                nc.scalar.activation(rz[:, :], rz_ps[:, :], AF.Tanh, scale=0.5)
                nc.vector.tensor_scalar(out=rz[:, :], in0=rz[:, :], scalar1=0.5,
                                        scalar2=0.5, op0=OP.mult, op1=OP.add)
                t1 = wk.tile([BC, H], F32, tag="t1")
                nc.vector.tensor_tensor(out=t1[:, :], in0=rz[:, 0:512], in1=hn_ps[:, :], op=OP.mult)
                nc.vector.tensor_tensor(out=t1[:, :], in0=t1[:, :], in1=in_ps[:, :], op=OP.add)
                nc.scalar.activation(t1[:, :], t1[:, :], AF.Tanh)   # t1 = n
                d_t = wk.tile([BC, H], F32, tag="d_t")
                nc.vector.tensor_tensor(out=d_t[:, :], in0=h_b[:, :], in1=t1[:, :], op=OP.subtract)
                nc.vector.tensor_tensor(out=d_t[:, :], in0=rz[:, 512:1024], in1=d_t[:, :], op=OP.mult)
                h_new = stp.tile([BC, H], F32, tag="h_b")
                nc.vector.tensor_tensor(out=h_new[:, :], in0=t1[:, :], in1=d_t[:, :], op=OP.add)
                h_b = h_new

                # hT update: 4 PE transposes [8,128] -> [128,8]
                tps = psS.tile([128, 4 * BC], F32, space="PSUM", tag="s")
                for hc in range(4):
                    nc.tensor.transpose(tps[:, hc * BC:(hc + 1) * BC],
                                        h_new[:, hc * 128:(hc + 1) * 128], ID8)
                hT_new = stp.tile([128, 4 * BC], F32, tag="hT")
                nc.vector.tensor_copy(hT_new[:, :], tps[:, :])
                hT = hT_new

                # ============ attention ============
                # cross[b,(bp,s)] = sum_h hT[h,b] * enc_proj[bp][h,s]; 4 K-chunks
                cr_ps = psS.tile([BC, BC * S], F32, space="PSUM", tag="s")
                for hc in range(4):
                    nc.tensor.matmul(
                        cr_ps[:, :], hT[:, hc * BC:(hc + 1) * BC],
                        epT[:, hc * BC * S:(hc + 1) * BC * S],
                        start=(hc == 0), stop=(hc == 3))
                # mask to the diagonal block and reduce over bp
                crm = wk.tile([BC, BC * S], F32, tag="crm")
                nc.vector.tensor_tensor(
                    out=_ap(crm, 0, [[BC * S, BC], [S, BC], [1, S]]),
                    in0=_ap(cr_ps, 0, [[BC * S, BC], [S, BC], [1, S]]),
                    in1=_ap(ident, 0, [[64, BC], [1, BC], [0, S]]),
                    op=OP.mult)
                sc_sb = wk.tile([BC, S], F32, tag="sc_sb")
                nc.vector.tensor_reduce(
                    _ap(sc_sb, 0, [[S, BC], [1, S]]),
                    _ap(crm, 0, [[BC * S, BC], [1, S], [S, BC]]),
                    axis=AX.X, op=OP.add)

                # softmax over s
                nmx = wk.tile([BC, 1], F32, tag="nmx")
                nc.vector.reduce_max(nmx[:, :], sc_sb[:, :], axis=AX.X)
                nc.vector.tensor_scalar_mul(nmx[:, :], nmx[:, :], -1.0)
                aw = wk.tile([BC, S], F32, tag="aw")
                sume = wk.tile([BC, 1], F32, tag="sume")
                nc.scalar.activation(aw[:, :], sc_sb[:, :], AF.Exp,
                                     bias=nmx[:, :], scale=1.0, accum_out=sume[:, :])
                rec = wk.tile([BC, 1], F32, tag="rec")
                nc.vector.reciprocal(rec[:, :], sume[:, :])
                nc.vector.tensor_scalar(out=aw[:, :], in0=aw[:, :], scalar1=rec[:, :],
                                        scalar2=None, op0=OP.mult)

                # awT + mask
                awT_ps = psS.tile([S, BC], F32, space="PSUM", tag="s")
                nc.tensor.transpose(awT_ps[:, :], aw[:, :], ID8)
                awT = wk.tile([S, BC], F32, tag="awT")
                nc.vector.tensor_copy(awT[:, :], awT_ps[:, :])
                awm = wk.tile([S, BC * BC], F32, tag="awm")
                nc.vector.tensor_tensor(
                    out=_ap(awm, 0, [[BC * BC, S], [BC, BC], [1, BC]]),
                    in0=_ap(awT, 0, [[BC, S], [1, BC], [0, BC]]),
                    in1=_ap(eye8, 0, [[BC * BC, S], [BC, BC], [1, BC]]),
                    op=OP.mult)

                # concat_out = tanh(ctx @ Wa_c.T + h @ Wa_h.T)
                cat_ps = psCAT.tile([BC, H], F32, space="PSUM", tag="catps")
                for bp in range(BC):
                    lhsT = _ap(awm, bp, [[BC * BC, S], [BC, BC]])
                    nc.tensor.matmul(cat_ps[:, :], lhsT,
                                     eoWs[0:S, bp * H:(bp + 1) * H],
                                     start=(bp == 0), stop=False)
                for hc in range(4):
                    nc.tensor.matmul(cat_ps[:, :], hT[:, hc * BC:(hc + 1) * BC],
                                     WahK[:, hc * H:(hc + 1) * H],
                                     start=False, stop=(hc == 3))
                co = wk.tile([BC, H], F32, tag="co")
                nc.scalar.activation(co[:, :], cat_ps[:, :], AF.Tanh)

                # ============ AllGather #1: concat_out ============
                nc.sync.dma_start(cc1_in.ap(), co[:, :])
                nc.gpsimd.collective_compute(
                    "AllGather", OP.bypass, replica_groups=rgroups,
                    ins=[cc1_in.ap().opt()], outs=[cc1_out.ap().opt()])
                warm_pe(26)
                coall = wk.tile([B, H], F32, tag="coall")
                nc.sync.dma_start(coall[:, :], cc1_out.ap())

                # concatT: [64,512] -> [128, 4*64]
                cT_ps = psS.tile([128, 4 * 64], F32, space="PSUM", tag="s")
                for hc in range(4):
                    nc.tensor.transpose(cT_ps[:, hc * 64:(hc + 1) * 64],
                                        coall[:, hc * 128:(hc + 1) * 128], ID64)
                cT = wk.tile([128, 4 * 64], F32, tag="cT")
                nc.vector.tensor_copy(cT[:, :], cT_ps[:, :])

                # ============ fc + per-bank stats ============
                staged = stg.tile([128, STG_W], F32, tag="staged")
                bmax8 = st.tile([128, NBANK * 8], F32, tag="bmax8")
                bidx8 = st.tile([128, NBANK * 8], U32, tag="bidx8")
                sume_b = st.tile([128, NBANK], F32, tag="sume_b")
                nc.vector.memset(bmax8[64:128, 48:56], -BIG)
                nc.vector.memset(bidx8[64:128, 48:56], 0)
                nc.vector.memset(sume_b[64:128, 6:7], 0.0)

                for bank in range(NBANK):
                    fc_ps = psFC.tile([128, 512], F32, space="PSUM", tag="fcps")
                    for hf in range(2):
                        tt_i = 2 * bank + hf
                        if tt_i >= NTILE:
                            continue
                        w = 512 if tt_i < 12 else TAIL_W
                        pr = fc_ps[64 * hf:64 * hf + 64, 0:w]
                        for kc in range(4):
                            nc.tensor.matmul(
                                pr,
                                cT[:, kc * 64:(kc + 1) * 64],
                                fcT[:, kc * VC + tt_i * 512: kc * VC + tt_i * 512 + w],
                                start=(kc == 0), stop=(kc == 3),
                                tile_position=(0, 64) if hf else None)
                    pmax = 128 if bank < 6 else 64
                    w_eff = 512 if bank < 6 else TAIL_W
                    s_sl = staged[0:pmax, bank * 512: bank * 512 + w_eff]
                    nc.scalar.copy(s_sl, fc_ps[0:pmax, 0:w_eff])
                    # exp in place on the psum bank (raw values already staged)
                    nc.scalar.activation(fc_ps[0:pmax, 0:w_eff], fc_ps[0:pmax, 0:w_eff],
                                         AF.Exp, accum_out=sume_b[0:pmax, bank:bank + 1])
                    nc.vector.max(out=bmax8[0:pmax, bank * 8:(bank + 1) * 8], in_=s_sl)
                    nc.vector.max_index(out=bidx8[0:pmax, bank * 8:(bank + 1) * 8],
                                        in_max=bmax8[0:pmax, bank * 8:(bank + 1) * 8],
                                        in_values=s_sl)

                # ============ local stats combine ============
                stats = st.tile([128, 4], F32, tag="stats")
                nc.vector.reduce_max(stats[:, 0:1], bmax8[:, :], axis=AX.X)
                nc.vector.reduce_sum(stats[:, 1:2], sume_b[:, :], axis=AX.X)
                cidx = st.tile([128, NBANK * 8], F32, tag="cidx")
                nc.vector.tensor_copy(cidx[:, :], bidx8[:, :])
                nc.vector.tensor_tensor(out=cidx[:, :], in0=cidx[:, :], in1=bankofs[:, :], op=OP.add)
                eqm = st.tile([128, NBANK * 8], F32, tag="eqm")
                nc.vector.tensor_tensor(out=eqm[:, :], in0=bmax8[:, :],
                                        in1=_ap(stats, 0, [[4, 128], [0, NBANK * 8]]),
                                        op=OP.is_equal)
                nc.vector.tensor_scalar(out=eqm[:, :], in0=eqm[:, :], scalar1=-BIG,
                                        scalar2=BIG, op0=OP.mult, op1=OP.add)
                nc.vector.tensor_tensor(out=cidx[:, :], in0=cidx[:, :], in1=eqm[:, :], op=OP.add)
                nc.vector.tensor_reduce(stats[:, 2:3], cidx[:, :], axis=AX.X, op=OP.min)
                nc.vector.memset(stats[:, 3:4], 0.0)

                # ============ AllGather #2: stats ============
                nc.sync.dma_start(cc2_in.ap(), stats[:, :])
                nc.gpsimd.collective_compute(
                    "AllGather", OP.bypass, replica_groups=rgroups,
                    ins=[cc2_in.ap().opt()], outs=[cc2_out.ap().opt()])
                warm_pe(34)
                gath = st.tile([128, 16 * 4], F32, tag="gath")
                gsrc = _ap(cc2_out, 0, [[4, 64], [512, NC], [256, 2], [1, 4]])
                nc.sync.dma_start(gath[0:64, :], gsrc)

                # global combine: token path first, on partitions 0:64 only
                lmaxs64 = _ap(gath, 0, [[64, 64], [4, 16]])
                gmax = st.tile([128, 1], F32, tag="gmax")
                nc.vector.reduce_max(gmax[0:64, :], lmaxs64, axis=AX.X)

                if t < t_steps - 1:
                    eq2 = st.tile([64, 16], F32, tag="eq2")
                    nc.vector.tensor_tensor(out=eq2[:, :], in0=lmaxs64,
                                            in1=_ap(gmax, 0, [[1, 64], [0, 16]]),
                                            op=OP.is_equal)
                    nc.vector.tensor_scalar(out=eq2[:, :], in0=eq2[:, :], scalar1=-BIG,
                                            scalar2=BIG, op0=OP.mult, op1=OP.add)
                    nc.vector.tensor_tensor(out=eq2[:, :], in0=eq2[:, :],
                                            in1=_ap(gath, 2, [[64, 64], [4, 16]]), op=OP.add)
                    tokf = st.tile([64, 1], F32, tag="tokf")
                    nc.vector.tensor_reduce(tokf[:, :], eq2[:, :], axis=AX.X, op=OP.min)
                    nc.vector.tensor_copy(toks[:, t + 1:t + 2], tokf[0:B, :])

                    # my 8 tokens -> indices -> embedding gather -> xT
                    tok_ps = psS.tile([BC, 1], F32, space="PSUM", tag="s")
                    nc.tensor.matmul(tok_ps[:, :], sel[:, :], tokf[:, :],
                                     start=True, stop=True)
                    tok_i = wk.tile([BC, 1], I32, tag="toki")
                    nc.vector.tensor_copy(tok_i[:, :], tok_ps[:, :])
                    x_g = wk.tile([BC, E], F32, tag="xg")
                    nc.gpsimd.indirect_dma_start(
                        out=x_g[:, :], out_offset=None, in_=emb_d.ap(),
                        in_offset=bass.IndirectOffsetOnAxis(ap=tok_i[:, 0:1], axis=0))
                    xt_ps = psS.tile([128, 2 * BC], F32, space="PSUM", tag="s")
                    for ec in range(2):
                        nc.tensor.transpose(xt_ps[:, ec * BC:(ec + 1) * BC],
                                            x_g[:, ec * 128:(ec + 1) * 128], ID8)
                    xT_new = stp.tile([128, 2 * BC], F32, tag="xT")
                    nc.vector.tensor_copy(xT_new[:, :], xt_ps[:, :])
                    xT = xT_new

                # per-shard sumexps were computed without max subtraction
                # (logits are O(1)), so lse = ln(sum_j sume_j) directly
                nc.sync.dma_start(gath[64:128, :], gsrc)
                ssum = st.tile([128, 1], F32, tag="ssum")
                nc.vector.reduce_sum(ssum[:, :], _ap(gath, 1, [[64, 128], [4, 16]]),
                                     axis=AX.X)

                # ln(ssum) via exponent bits + quadratic + 2 Newton steps (Exp only)
                si = ssum[:, :].bitcast(I32)
                e_i = st.tile([128, 1], I32, tag="e_i")
                nc.vector.tensor_scalar(out=e_i[:, :], in0=si, scalar1=23,
                                        scalar2=None, op0=OP.arith_shift_right)
                e_f = st.tile([128, 1], F32, tag="e_f")
                nc.vector.tensor_copy(e_f[:, :], e_i[:, :])
                nc.vector.tensor_scalar(out=e_f[:, :], in0=e_f[:, :], scalar1=-127.0,
                                        scalar2=None, op0=OP.add)
                m_i = st.tile([128, 1], I32, tag="m_i")
                nc.vector.tensor_scalar(out=m_i[:, :], in0=si, scalar1=0x7FFFFF,
                                        scalar2=None, op0=OP.bitwise_and)
                nc.vector.tensor_scalar(out=m_i[:, :], in0=m_i[:, :], scalar1=0x3F800000,
                                        scalar2=None, op0=OP.bitwise_or)
                m_f = m_i[:, :].bitcast(F32)
                poly = st.tile([128, 1], F32, tag="poly")
                nc.vector.tensor_scalar(out=poly[:, :], in0=m_f, scalar1=LC2,
                                        scalar2=LC1, op0=OP.mult, op1=OP.add)
                nc.vector.tensor_tensor(out=poly[:, :], in0=poly[:, :], in1=m_f, op=OP.mult)
                nc.vector.tensor_scalar(out=poly[:, :], in0=poly[:, :], scalar1=LC0,
                                        scalar2=None, op0=OP.add)
                lnv = st.tile([128, 1], F32, tag="lnv")
                nc.vector.tensor_tensor(out=lnv[:, :], in0=poly[:, :], in1=e_f[:, :], op=OP.add)
                nc.vector.tensor_scalar(out=lnv[:, :], in0=lnv[:, :], scalar1=LN2,
                                        scalar2=None, op0=OP.mult)
                for _ in range(2):
                    nx = st.tile([128, 1], F32, tag="nx")
                    nc.scalar.activation(nx[:, :], lnv[:, :], AF.Exp, scale=-1.0)
                    nc.vector.tensor_tensor(out=nx[:, :], in0=nx[:, :], in1=ssum[:, :], op=OP.mult)
                    nc.vector.tensor_scalar(out=nx[:, :], in0=nx[:, :], scalar1=-1.0,
                                            scalar2=None, op0=OP.add)
                    nc.vector.tensor_tensor(out=lnv[:, :], in0=lnv[:, :], in1=nx[:, :], op=OP.add)

                nlse = st.tile([128, 1], F32, tag="nlse")
                nc.vector.tensor_scalar_mul(nlse[:, :], lnv[:, :], -1.0)

                # subtract lse in place, then write out
                nc.vector.tensor_scalar(out=staged[:, 0:3072], in0=staged[:, 0:3072],
                                        scalar1=nlse[:, :], scalar2=None, op0=OP.add)
                nc.vector.tensor_scalar(out=staged[0:64, 3072:STG_W],
                                        in0=staged[0:64, 3072:STG_W],
                                        scalar1=nlse[0:64, :], scalar2=None, op0=OP.add)
                dst0 = _ap(olog_d, t * VC, [[t_steps * VC, B], [1024, 6], [1, 512]])
                nc.sync.dma_start(dst0, staged[0:64, 0:3072].rearrange("p (a b) -> p a b", a=6))
                dst1 = _ap(olog_d, t * VC + 512, [[t_steps * VC, B], [1024, 6], [1, 512]])
                nc.sync.dma_start(dst1, staged[64:128, 0:3072].rearrange("p (a b) -> p a b", a=6))
                dst2 = _ap(olog_d, t * VC + 6144, [[t_steps * VC, B], [1, TAIL_W]])
                nc.sync.dma_start(dst2, staged[0:64, 3072:3072 + TAIL_W])

            # epilogue
            nc.sync.dma_start(oh_d.ap(), h_b[:, :])
            nc.vector.memset(toks[:, 0:1], float(SOS))
            nc.sync.dma_start(otok_d.ap(), toks[:, :])

    nc.compile()
    return nc


# ======================= host side =======================

_CACHE = {}


def _prep_inputs(inputs, t_steps=T):
    emb = np.ascontiguousarray(np.asarray(inputs["embedding"], dtype=np.float32))
    eh = np.asarray(inputs["encoder_hidden"], dtype=np.float32)
    eo = np.asarray(inputs["encoder_outputs"], dtype=np.float32)
    w_ih = np.asarray(inputs["w_ih"], dtype=np.float32)
    w_hh = np.asarray(inputs["w_hh"], dtype=np.float32)
    attn_w = np.asarray(inputs["attn_w"], dtype=np.float32)
    Wa = np.asarray(inputs["Wa"], dtype=np.float32)
    fc_w = np.asarray(inputs["fc_w"], dtype=np.float32)

    h0 = eh[0]                                   # [B, H]
    x0 = emb[SOS]                                # [E]
    x0T = np.zeros((128, 2 * BC), np.float32)
    for ec in range(2):
        x0T[:, ec * BC:(ec + 1) * BC] = np.repeat(
            x0[ec * 128:(ec + 1) * 128][:, None], BC, 1)
    wihK = w_ih.T.reshape(2, 128, 3 * H).transpose(1, 0, 2).reshape(128, 2 * 3 * H)
    whhK = w_hh.T.reshape(4, 128, 3 * H).transpose(1, 0, 2).reshape(128, 4 * 3 * H)
    Wah = Wa[:, 0:H]                             # [512g, 512h]
    WahK = Wah.T.reshape(4, 128, H).transpose(1, 0, 2).reshape(128, 4 * H)
    Wac = Wa[:, H:2 * H]
    eye8 = np.tile(np.eye(BC, dtype=np.float32).reshape(1, BC * BC), (128, 1))
    base_b = np.zeros((128, NBANK * 8), np.float32)
    for bank in range(NBANK):
        base_b[:, bank * 8:(bank + 1) * 8] = bank * 1024
    base_b[64:128, :] += 512

    in_maps = []
    for c in range(NC):
        bs = slice(c * BC, (c + 1) * BC)
        eo_c = eo[bs]                            # [BC, S, H]
        h0b = np.ascontiguousarray(h0[bs])
        h0T = h0b.T.reshape(4, 128, BC).transpose(1, 0, 2).reshape(128, 4 * BC)
        # epT[p, (gc*8+b)*64+s] = ep_c[b, s, gc*128+p], ep = eo @ attn_w.T
        ep_c = np.einsum("bsh,gh->bsg", eo_c, attn_w).astype(np.float32)
        epT = ep_c.transpose(2, 0, 1).reshape(4, 128, BC, S).transpose(
            1, 0, 2, 3).reshape(128, 4 * BC * S)
        eoW = np.einsum("bsh,gh->bsg", eo_c, Wac).astype(np.float32)
        eoWs = eoW.transpose(1, 0, 2).reshape(S, BC * H)
        fc_c = fc_w[c * VC:(c + 1) * VC]         # [VC, H]
        fcT = fc_c.T.reshape(4, 128, VC).transpose(1, 0, 2).reshape(128, 4 * VC)
        sel_m = np.zeros((B, BC), np.float32)
        for j in range(BC):
            sel_m[c * BC + j, j] = 1.0
        bofs = base_b + c * VC
        in_maps.append({
            "emb": emb, "x0T": x0T, "h0b": h0b, "h0T": h0T,
            "wihK": np.ascontiguousarray(wihK), "whhK": np.ascontiguousarray(whhK),
            "WahK": np.ascontiguousarray(WahK),
            "epT": np.ascontiguousarray(epT), "eoWs": np.ascontiguousarray(eoWs),
            "fcT": np.ascontiguousarray(fcT), "sel": sel_m,
            "eye8r": eye8, "bankofs": bofs.astype(np.float32),
        })
    return in_maps


def kernel(**inputs):
    if "nc" not in _CACHE:
        _CACHE["nc"] = build(T)
    nc = _CACHE["nc"]
    in_maps = _prep_inputs(inputs, T)
    res = bass_utils.run_bass_kernel_spmd(nc, in_maps, core_ids=list(range(NC)))
    outs = res.results
    dec = np.concatenate([outs[c]["o_logits"] for c in range(NC)], axis=2)
    h_fin = np.concatenate([outs[c]["o_h"] for c in range(NC)], axis=0)[None]
    return dec, h_fin


# revision 16
# speedup vs baseline: 1.1988x; 1.1988x over previous
"""Trainium2 Bass kernel for nn_Decoder (GRU + Luong attention + greedy decode).

Sharding: hybrid
  - GRU + attention: data-parallel over batch (8 rows/core)
  - fc ([V,H] @ h) + log-softmax stats + argmax: tensor-parallel over vocab
    (6250 rows/core, fc_w slice resident in SBUF)
  - two small AllGathers per step: concat_out ([8,512]->[64,512]) and
    per-shard (max, sumexp, argmax) stats ([128,4]->[1024,4])

Everything on the argmax feedback path is fp32 (top1-top2 logit gaps go down
to 3.5e-5 on these inputs; any bf16 would flip tokens and cascade).
"""

import numpy as np

import concourse.bacc as bacc
import concourse.bass as bass
import concourse.mybir as mybir
import concourse.tile as tile
from concourse.masks import make_identity
from concourse import bass_utils

F32 = mybir.dt.float32
I32 = mybir.dt.int32
U32 = mybir.dt.uint32
AF = mybir.ActivationFunctionType
OP = mybir.AluOpType
AX = mybir.AxisListType

NC = 8          # cores
B = 64          # batch
BC = B // NC    # batch rows per core
S = 64          # source length
E = 256         # embed dim
H = 512         # hidden
V = 50000       # vocab
VC = V // NC    # vocab rows per core
T = 32          # decode steps
SOS = 1

NTILE = 13          # vocab tiles per core: 12 x 512 + 1 x 106
TAIL_W = VC - 12 * 512   # 106
NBANK = 7           # fc psum passes (2 tiles/bank, bank 6 half-used)
STG_W = 6 * 512 + TAIL_W  # staged logits width per partition (3178)
BIG = 1.0e9

LN2 = 0.6931471805599453
# quadratic fit for log2(m), m in [1,2); refined by 2 Newton steps w/ Exp
LC2, LC1, LC0 = -0.344845, 2.024658, -1.674873


def _ap(t, off, dims):
    """Hand-built access pattern view: dims = [[stride, size], ...] (elements)."""
    tensor = t.tensor if isinstance(t, bass.AP) else t
    return bass.AP(tensor, off, dims)


def build(t_steps=T):
    nc = bacc.Bacc("TRN2", target_bir_lowering=False, debug=False, num_devices=NC)

    # ---------------- external inputs (per core) ----------------
    emb_d = nc.dram_tensor("emb", [V, E], F32, kind="ExternalInput")
    x0T_d = nc.dram_tensor("x0T", [128, 2 * BC], F32, kind="ExternalInput")
    h0b_d = nc.dram_tensor("h0b", [BC, H], F32, kind="ExternalInput")
    h0T_d = nc.dram_tensor("h0T", [128, 4 * BC], F32, kind="ExternalInput")
    wih_d = nc.dram_tensor("wihK", [128, 2 * 3 * H], F32, kind="ExternalInput")
    whh_d = nc.dram_tensor("whhK", [128, 4 * 3 * H], F32, kind="ExternalInput")
    WahK_d = nc.dram_tensor("WahK", [128, 4 * H], F32, kind="ExternalInput")
    epT_d = nc.dram_tensor("epT", [128, 4 * BC * S], F32, kind="ExternalInput")
    eoWs_d = nc.dram_tensor("eoWs", [S, BC * H], F32, kind="ExternalInput")
    fcT_d = nc.dram_tensor("fcT", [128, 4 * VC], F32, kind="ExternalInput")
    sel_d = nc.dram_tensor("sel", [B, BC], F32, kind="ExternalInput")
    eye_d = nc.dram_tensor("eye8r", [128, BC * BC], F32, kind="ExternalInput")
    bofs_d = nc.dram_tensor("bankofs", [128, NBANK * 8], F32, kind="ExternalInput")

    # ---------------- external outputs (per core) ----------------
    olog_d = nc.dram_tensor("o_logits", [B, t_steps, VC], F32, kind="ExternalOutput")
    oh_d = nc.dram_tensor("o_h", [BC, H], F32, kind="ExternalOutput")
    otok_d = nc.dram_tensor("o_tok", [B, t_steps], F32, kind="ExternalOutput")

    # ---------------- internal dram (collective bounce) ----------------
    cc1_in = nc.dram_tensor("cc1_in", [BC, H], F32)
    cc1_out = nc.dram_tensor("cc1_out", [B, H], F32, addr_space="Shared")
    cc2_in = nc.dram_tensor("cc2_in", [128, 4], F32)
    cc2_out = nc.dram_tensor("cc2_out", [128 * NC, 4], F32, addr_space="Shared")
    rgroups = [list(range(NC))]

    with tile.TileContext(nc) as tc:
        with tc.tile_pool(name="res", bufs=1) as res, \
             tc.tile_pool(name="state", bufs=1) as stp, \
             tc.tile_pool(name="work", bufs=1) as wk, \
             tc.tile_pool(name="stats", bufs=1) as st, \
             tc.tile_pool(name="stg", bufs=1) as stg, \
             tc.tile_pool(name="psRZ", bufs=1, space="PSUM") as psRZ, \
             tc.tile_pool(name="psCAT", bufs=1, space="PSUM") as psCAT, \
             tc.tile_pool(name="psS", bufs=2, space="PSUM") as psS, \
             tc.tile_pool(name="psDUM", bufs=1, space="PSUM") as psDUM, \
             tc.tile_pool(name="psFC", bufs=2, space="PSUM") as psFC:

            # ---------------- residents ----------------
            ident = res.tile([64, 64], F32)
            make_identity(nc, ident[:, :])
            wih = res.tile([128, 2 * 3 * H], F32)
            nc.sync.dma_start(wih[:, :], wih_d.ap())
            whh = res.tile([128, 4 * 3 * H], F32)
            nc.sync.dma_start(whh[:, :], whh_d.ap())
            WahK = res.tile([128, 4 * H], F32)
            nc.sync.dma_start(WahK[:, :], WahK_d.ap())
            epT = res.tile([128, 4 * BC * S], F32)
            nc.sync.dma_start(epT[:, :], epT_d.ap())
            dumW = res.tile([128, 64], F32)
            nc.gpsimd.memset(dumW[:, :], 0.001)
            eoWs = res.tile([S, BC * H], F32)
            nc.sync.dma_start(eoWs[:, :], eoWs_d.ap())
            fcT = res.tile([128, 4 * VC], F32)
            nc.sync.dma_start(fcT[:, :], fcT_d.ap())
            sel = res.tile([B, BC], F32)
            nc.sync.dma_start(sel[:, :], sel_d.ap())
            eye8 = res.tile([128, BC * BC], F32)
            nc.sync.dma_start(eye8[:, :], eye_d.ap())
            bankofs = res.tile([128, NBANK * 8], F32)
            nc.sync.dma_start(bankofs[:, :], bofs_d.ap())
            toks = res.tile([B, t_steps], F32)

            # initial state
            xT = stp.tile([128, 2 * BC], F32, tag="xT")
            nc.sync.dma_start(xT[:, :], x0T_d.ap())
            h_b = stp.tile([BC, H], F32, tag="h_b")
            nc.sync.dma_start(h_b[:, :], h0b_d.ap())
            hT = stp.tile([128, 4 * BC], F32, tag="hT")
            nc.sync.dma_start(hT[:, :], h0T_d.ap())

            ID8 = ident[0:BC, 0:BC]
            ID64 = ident[0:64, 0:64]

            dum_ps = psDUM.tile([128, 512], F32, space="PSUM", tag="dum")
            nc.tensor.matmul(dum_ps[0:64, :], dumW[:, 0:64], wih[:, 0:512],
                             start=True, stop=False, skip_group_check=True)

            def warm_pe(n_mm):
                for _ in range(n_mm):
                    nc.tensor.matmul(dum_ps[0:64, :], dumW[:, 0:64], wih[:, 0:512],
                                     start=False, stop=False, skip_group_check=True)

            for t in range(t_steps):
                # ============ GRU ============
                rz_ps = psRZ.tile([BC, 1024], F32, space="PSUM", tag="rz")
                in_ps = psS.tile([BC, H], F32, space="PSUM", tag="s")
                hn_ps = psS.tile([BC, H], F32, space="PSUM", tag="s")
                # gh contributions first (only need hT -> overlap the token path)
                for hc in range(4):
                    for nt in range(2):
                        sl = slice(nt * 512, nt * 512 + 512)
                        nc.tensor.matmul(
                            rz_ps[:, sl], hT[:, hc * BC:(hc + 1) * BC],
                            whh[:, hc * 1536 + nt * 512: hc * 1536 + nt * 512 + 512],
                            start=(hc == 0), stop=False)
                    nc.tensor.matmul(
                        hn_ps[:, :], hT[:, hc * BC:(hc + 1) * BC],
                        whh[:, hc * 1536 + 1024: hc * 1536 + 1536],
                        start=(hc == 0), stop=(hc == 3))
                # gi contributions (need xT of this step)
                for ec in range(2):
                    for nt in range(2):
                        sl = slice(nt * 512, nt * 512 + 512)
                        nc.tensor.matmul(
                            rz_ps[:, sl], xT[:, ec * BC:(ec + 1) * BC],
                            wih[:, ec * 1536 + nt * 512: ec * 1536 + nt * 512 + 512],
                            start=False, stop=(ec == 1))
                    nc.tensor.matmul(
                        in_ps[:, :], xT[:, ec * BC:(ec + 1) * BC],
                        wih[:, ec * 1536 + 1024: ec * 1536 + 1536],
                        start=(ec == 0), stop=(ec == 1))

                # gates: sigmoid(x) = 0.5*tanh(0.5x)+0.5 (stay in exp/tanh table set)
                rz = wk.tile([BC, 1024], F32, tag="rz_sb")
                nc.scalar.activation(rz[:, :], rz_ps[:, :], AF.Tanh, scale=0.5)
                nc.vector.tensor_scalar(out=rz[:, :], in0=rz[:, :], scalar1=0.5,
                                        scalar2=0.5, op0=OP.mult, op1=OP.add)
                t1 = wk.tile([BC, H], F32, tag="t1")
                nc.vector.tensor_tensor(out=t1[:, :], in0=rz[:, 0:512], in1=hn_ps[:, :], op=OP.mult)
                nc.vector.tensor_tensor(out=t1[:, :], in0=t1[:, :], in1=in_ps[:, :], op=OP.add)
                nc.scalar.activation(t1[:, :], t1[:, :], AF.Tanh)   # t1 = n
                d_t = wk.tile([BC, H], F32, tag="d_t")
                nc.vector.tensor_tensor(out=d_t[:, :], in0=h_b[:, :], in1=t1[:, :], op=OP.subtract)
                nc.vector.tensor_tensor(out=d_t[:, :], in0=rz[:, 512:1024], in1=d_t[:, :], op=OP.mult)
                h_new = stp.tile([BC, H], F32, tag="h_b")
                nc.vector.tensor_tensor(out=h_new[:, :], in0=t1[:, :], in1=d_t[:, :], op=OP.add)
                h_b = h_new

                # hT update: 4 PE transposes [8,128] -> [128,8]
                tps = psS.tile([128, 4 * BC], F32, space="PSUM", tag="s")
                for hc in range(4):
                    nc.tensor.transpose(tps[:, hc * BC:(hc + 1) * BC],
                                        h_new[:, hc * 128:(hc + 1) * 128], ID8)
                hT_new = stp.tile([128, 4 * BC], F32, tag="hT")
                nc.vector.tensor_copy(hT_new[:, :], tps[:, :])
                hT = hT_new

                # ============ attention ============
                # cross[b,(bp,s)] = sum_h hT[h,b] * enc_proj[bp][h,s]; 4 K-chunks
                cr_ps = psS.tile([BC, BC * S], F32, space="PSUM", tag="s")
                for hc in range(4):
                    nc.tensor.matmul(
                        cr_ps[:, :], hT[:, hc * BC:(hc + 1) * BC],
                        epT[:, hc * BC * S:(hc + 1) * BC * S],
                        start=(hc == 0), stop=(hc == 3))
                # mask to the diagonal block and reduce over bp
                crm = wk.tile([BC, BC * S], F32, tag="crm")
                nc.vector.tensor_tensor(
                    out=_ap(crm, 0, [[BC * S, BC], [S, BC], [1, S]]),
                    in0=_ap(cr_ps, 0, [[BC * S, BC], [S, BC], [1, S]]),
                    in1=_ap(ident, 0, [[64, BC], [1, BC], [0, S]]),
                    op=OP.mult)
                sc_sb = wk.tile([BC, S], F32, tag="sc_sb")
                nc.vector.tensor_reduce(
                    _ap(sc_sb, 0, [[S, BC], [1, S]]),
                    _ap(crm, 0, [[BC * S, BC], [1, S], [S, BC]]),
                    axis=AX.X, op=OP.add)

                # softmax over s
                nmx = wk.tile([BC, 1], F32, tag="nmx")
                nc.vector.reduce_max(nmx[:, :], sc_sb[:, :], axis=AX.X)
                nc.vector.tensor_scalar_mul(nmx[:, :], nmx[:, :], -1.0)
                aw = wk.tile([BC, S], F32, tag="aw")
                sume = wk.tile([BC, 1], F32, tag="sume")
                nc.scalar.activation(aw[:, :], sc_sb[:, :], AF.Exp,
                                     bias=nmx[:, :], scale=1.0, accum_out=sume[:, :])
                rec = wk.tile([BC, 1], F32, tag="rec")
                nc.vector.reciprocal(rec[:, :], sume[:, :])
                nc.vector.tensor_scalar(out=aw[:, :], in0=aw[:, :], scalar1=rec[:, :],
                                        scalar2=None, op0=OP.mult)

                # awT + mask
                awT_ps = psS.tile([S, BC], F32, space="PSUM", tag="s")
                nc.tensor.transpose(awT_ps[:, :], aw[:, :], ID8)
                awT = wk.tile([S, BC], F32, tag="awT")
                nc.vector.tensor_copy(awT[:, :], awT_ps[:, :])
                awm = wk.tile([S, BC * BC], F32, tag="awm")
                nc.vector.tensor_tensor(
                    out=_ap(awm, 0, [[BC * BC, S], [BC, BC], [1, BC]]),
                    in0=_ap(awT, 0, [[BC, S], [1, BC], [0, BC]]),
                    in1=_ap(eye8, 0, [[BC * BC, S], [BC, BC], [1, BC]]),
                    op=OP.mult)

                # concat_out = tanh(ctx @ Wa_c.T + h @ Wa_h.T)
                cat_ps = psCAT.tile([BC, H], F32, space="PSUM", tag="catps")
                for bp in range(BC):
                    lhsT = _ap(awm, bp, [[BC * BC, S], [BC, BC]])
                    nc.tensor.matmul(cat_ps[:, :], lhsT,
                                     eoWs[0:S, bp * H:(bp + 1) * H],
                                     start=(bp == 0), stop=False)
                for hc in range(4):
                    nc.tensor.matmul(cat_ps[:, :], hT[:, hc * BC:(hc + 1) * BC],
                                     WahK[:, hc * H:(hc + 1) * H],
                                     start=False, stop=(hc == 3))
                co = wk.tile([BC, H], F32, tag="co")
                nc.scalar.activation(co[:, :], cat_ps[:, :], AF.Tanh)

                # ============ AllGather #1: concat_out ============
                nc.sync.dma_start(cc1_in.ap(), co[:, :])
                nc.gpsimd.collective_compute(
                    "AllGather", OP.bypass, replica_groups=rgroups,
                    ins=[cc1_in.ap().opt()], outs=[cc1_out.ap().opt()])
                coall = wk.tile([B, H], F32, tag="coall")
                nc.sync.dma_start(coall[:, :], cc1_out.ap())

                # concatT: [64,512] -> [128, 4*64]
                cT_ps = psS.tile([128, 4 * 64], F32, space="PSUM", tag="s")
                for hc in range(4):
                    nc.tensor.transpose(cT_ps[:, hc * 64:(hc + 1) * 64],
                                        coall[:, hc * 128:(hc + 1) * 128], ID64)
                cT = wk.tile([128, 4 * 64], F32, tag="cT")
                nc.vector.tensor_copy(cT[:, :], cT_ps[:, :])

                # ============ fc + per-bank stats ============
                staged = stg.tile([128, STG_W], F32, tag="staged")
                bmax8 = st.tile([128, NBANK * 8], F32, tag="bmax8")
                bidx8 = st.tile([128, NBANK * 8], U32, tag="bidx8")
                sume_b = st.tile([128, NBANK], F32, tag="sume_b")
                nc.vector.memset(bmax8[64:128, 48:56], -BIG)
                nc.vector.memset(bidx8[64:128, 48:56], 0)
                nc.vector.memset(sume_b[64:128, 6:7], 0.0)

                for bank in range(NBANK):
                    fc_ps = psFC.tile([128, 512], F32, space="PSUM", tag="fcps")
                    for kc in range(4):
                        for hf in range(2):
                            tt_i = 2 * bank + hf
                            if tt_i >= NTILE:
                                continue
                            w = 512 if tt_i < 12 else TAIL_W
                            pr = fc_ps[64 * hf:64 * hf + 64, 0:w]
                            nc.tensor.matmul(
                                pr,
                                cT[:, kc * 64:(kc + 1) * 64],
                                fcT[:, kc * VC + tt_i * 512: kc * VC + tt_i * 512 + w],
                                start=(kc == 0), stop=(kc == 3),
                                tile_position=(0, 64) if hf else None,
                                skip_group_check=True)
                    pmax = 128 if bank < 6 else 64
                    w_eff = 512 if bank < 6 else TAIL_W
                    s_sl = staged[0:pmax, bank * 512: bank * 512 + w_eff]
                    nc.scalar.copy(s_sl, fc_ps[0:pmax, 0:w_eff])
                    # exp in place on the psum bank (raw values already staged)
                    nc.scalar.activation(fc_ps[0:pmax, 0:w_eff], fc_ps[0:pmax, 0:w_eff],
                                         AF.Exp, accum_out=sume_b[0:pmax, bank:bank + 1])
                    nc.vector.max(out=bmax8[0:pmax, bank * 8:(bank + 1) * 8], in_=s_sl)
                    nc.vector.max_index(out=bidx8[0:pmax, bank * 8:(bank + 1) * 8],
                                        in_max=bmax8[0:pmax, bank * 8:(bank + 1) * 8],
                                        in_values=s_sl)

                # ============ local stats combine ============
                stats = st.tile([128, 4], F32, tag="stats")
                nc.vector.reduce_max(stats[:, 0:1], bmax8[:, :], axis=AX.X)
                nc.vector.reduce_sum(stats[:, 1:2], sume_b[:, :], axis=AX.X)
                cidx = st.tile([128, NBANK * 8], F32, tag="cidx")
                nc.vector.tensor_copy(cidx[:, :], bidx8[:, :])
                nc.vector.tensor_tensor(out=cidx[:, :], in0=cidx[:, :], in1=bankofs[:, :], op=OP.add)
                eqm = st.tile([128, NBANK * 8], F32, tag="eqm")
                nc.vector.tensor_tensor(out=eqm[:, :], in0=bmax8[:, :],
                                        in1=_ap(stats, 0, [[4, 128], [0, NBANK * 8]]),
                                        op=OP.is_equal)
                nc.vector.tensor_scalar(out=eqm[:, :], in0=eqm[:, :], scalar1=-BIG,
                                        scalar2=BIG, op0=OP.mult, op1=OP.add)
                nc.vector.tensor_tensor(out=cidx[:, :], in0=cidx[:, :], in1=eqm[:, :], op=OP.add)
                nc.vector.tensor_reduce(stats[:, 2:3], cidx[:, :], axis=AX.X, op=OP.min)
                nc.vector.memset(stats[:, 3:4], 0.0)

                # ============ AllGather #2: stats ============
                nc.sync.dma_start(cc2_in.ap(), stats[:, :])
                nc.gpsimd.collective_compute(
                    "AllGather", OP.bypass, replica_groups=rgroups,
                    ins=[cc2_in.ap().opt()], outs=[cc2_out.ap().opt()])
                gath = st.tile([128, 16 * 4], F32, tag="gath")
                gsrc = _ap(cc2_out, 0, [[4, 64], [512, NC], [256, 2], [1, 4]])
                nc.sync.dma_start(gath[0:64, :], gsrc)

                # global combine: token path first, on partitions 0:64 only
                lmaxs64 = _ap(gath, 0, [[64, 64], [4, 16]])
                gmax = st.tile([128, 1], F32, tag="gmax")
                nc.vector.reduce_max(gmax[0:64, :], lmaxs64, axis=AX.X)

                if t < t_steps - 1:
                    eq2 = st.tile([64, 16], F32, tag="eq2")
                    nc.vector.tensor_tensor(out=eq2[:, :], in0=lmaxs64,
                                            in1=_ap(gmax, 0, [[1, 64], [0, 16]]),
                                            op=OP.is_equal)
                    nc.vector.tensor_scalar(out=eq2[:, :], in0=eq2[:, :], scalar1=-BIG,
                                            scalar2=BIG, op0=OP.mult, op1=OP.add)
                    nc.vector.tensor_tensor(out=eq2[:, :], in0=eq2[:, :],
                                            in1=_ap(gath, 2, [[64, 64], [4, 16]]), op=OP.add)
                    tokf = st.tile([64, 1], F32, tag="tokf")
                    nc.vector.tensor_reduce(tokf[:, :], eq2[:, :], axis=AX.X, op=OP.min)
                    nc.vector.tensor_copy(toks[:, t + 1:t + 2], tokf[0:B, :])

                    # my 8 tokens -> indices -> embedding gather -> xT
                    tok_ps = psS.tile([BC, 1], F32, space="PSUM", tag="s")
                    nc.tensor.matmul(tok_ps[:, :], sel[:, :], tokf[:, :],
                                     start=True, stop=True)
                    tok_i = wk.tile([BC, 1], I32, tag="toki")
                    nc.vector.tensor_copy(tok_i[:, :], tok_ps[:, :])
                    x_g = wk.tile([BC, E], F32, tag="xg")
                    nc.gpsimd.indirect_dma_start(
                        out=x_g[:, :], out_offset=None, in_=emb_d.ap(),
                        in_offset=bass.IndirectOffsetOnAxis(ap=tok_i[:, 0:1], axis=0))
                    xt_ps = psS.tile([128, 2 * BC], F32, space="PSUM", tag="s")
                    for ec in range(2):
                        nc.tensor.transpose(xt_ps[:, ec * BC:(ec + 1) * BC],
                                            x_g[:, ec * 128:(ec + 1) * 128], ID8)
                    xT_new = stp.tile([128, 2 * BC], F32, tag="xT")
                    nc.vector.tensor_copy(xT_new[:, :], xt_ps[:, :])
                    xT = xT_new

                # per-shard sumexps were computed without max subtraction
                # (logits are O(1)), so lse = ln(sum_j sume_j) directly
                nc.sync.dma_start(gath[64:128, :], gsrc)
                ssum = st.tile([128, 1], F32, tag="ssum")
                nc.vector.reduce_sum(ssum[:, :], _ap(gath, 1, [[64, 128], [4, 16]]),
                                     axis=AX.X)

                # ln(ssum) via exponent bits + quadratic + 2 Newton steps (Exp only)
                si = ssum[:, :].bitcast(I32)
                e_i = st.tile([128, 1], I32, tag="e_i")
                nc.vector.tensor_scalar(out=e_i[:, :], in0=si, scalar1=23,
                                        scalar2=None, op0=OP.arith_shift_right)
                e_f = st.tile([128, 1], F32, tag="e_f")
                nc.vector.tensor_copy(e_f[:, :], e_i[:, :])
                nc.vector.tensor_scalar(out=e_f[:, :], in0=e_f[:, :], scalar1=-127.0,
                                        scalar2=None, op0=OP.add)
                m_i = st.tile([128, 1], I32, tag="m_i")
                nc.vector.tensor_scalar(out=m_i[:, :], in0=si, scalar1=0x7FFFFF,
                                        scalar2=None, op0=OP.bitwise_and)
                nc.vector.tensor_scalar(out=m_i[:, :], in0=m_i[:, :], scalar1=0x3F800000,
                                        scalar2=None, op0=OP.bitwise_or)
                m_f = m_i[:, :].bitcast(F32)
                poly = st.tile([128, 1], F32, tag="poly")
                nc.vector.tensor_scalar(out=poly[:, :], in0=m_f, scalar1=LC2,
                                        scalar2=LC1, op0=OP.mult, op1=OP.add)
                nc.vector.tensor_tensor(out=poly[:, :], in0=poly[:, :], in1=m_f, op=OP.mult)
                nc.vector.tensor_scalar(out=poly[:, :], in0=poly[:, :], scalar1=LC0,
                                        scalar2=None, op0=OP.add)
                lnv = st.tile([128, 1], F32, tag="lnv")
                nc.vector.tensor_tensor(out=lnv[:, :], in0=poly[:, :], in1=e_f[:, :], op=OP.add)
                nc.vector.tensor_scalar(out=lnv[:, :], in0=lnv[:, :], scalar1=LN2,
                                        scalar2=None, op0=OP.mult)
                for _ in range(2):
                    nx = st.tile([128, 1], F32, tag="nx")
                    nc.scalar.activation(nx[:, :], lnv[:, :], AF.Exp, scale=-1.0)
                    nc.vector.tensor_tensor(out=nx[:, :], in0=nx[:, :], in1=ssum[:, :], op=OP.mult)
                    nc.vector.tensor_scalar(out=nx[:, :], in0=nx[:, :], scalar1=-1.0,
                                            scalar2=None, op0=OP.add)
                    nc.vector.tensor_tensor(out=lnv[:, :], in0=lnv[:, :], in1=nx[:, :], op=OP.add)

                nlse = st.tile([128, 1], F32, tag="nlse")
                nc.vector.tensor_scalar_mul(nlse[:, :], lnv[:, :], -1.0)

                # subtract lse in place, then write out
                nc.vector.tensor_scalar(out=staged[:, 0:3072], in0=staged[:, 0:3072],
                                        scalar1=nlse[:, :], scalar2=None, op0=OP.add)
                nc.vector.tensor_scalar(out=staged[0:64, 3072:STG_W],
                                        in0=staged[0:64, 3072:STG_W],
                                        scalar1=nlse[0:64, :], scalar2=None, op0=OP.add)
                dst0 = _ap(olog_d, t * VC, [[t_steps * VC, B], [1024, 6], [1, 512]])
                nc.sync.dma_start(dst0, staged[0:64, 0:3072].rearrange("p (a b) -> p a b", a=6))
                dst1 = _ap(olog_d, t * VC + 512, [[t_steps * VC, B], [1024, 6], [1, 512]])
                nc.sync.dma_start(dst1, staged[64:128, 0:3072].rearrange("p (a b) -> p a b", a=6))
                dst2 = _ap(olog_d, t * VC + 6144, [[t_steps * VC, B], [1, TAIL_W]])
                nc.sync.dma_start(dst2, staged[0:64, 3072:3072 + TAIL_W])

            # epilogue
            nc.sync.dma_start(oh_d.ap(), h_b[:, :])
            nc.vector.memset(toks[:, 0:1], float(SOS))
            nc.sync.dma_start(otok_d.ap(), toks[:, :])

    nc.compile()
    return nc


# ======================= host side =======================

_CACHE = {}


def _prep_inputs(inputs, t_steps=T):
    emb = np.ascontiguousarray(np.asarray(inputs["embedding"], dtype=np.float32))
    eh = np.asarray(inputs["encoder_hidden"], dtype=np.float32)
    eo = np.asarray(inputs["encoder_outputs"], dtype=np.float32)
    w_ih = np.asarray(inputs["w_ih"], dtype=np.float32)
    w_hh = np.asarray(inputs["w_hh"], dtype=np.float32)
    attn_w = np.asarray(inputs["attn_w"], dtype=np.float32)
    Wa = np.asarray(inputs["Wa"], dtype=np.float32)
    fc_w = np.asarray(inputs["fc_w"], dtype=np.float32)

    h0 = eh[0]                                   # [B, H]
    x0 = emb[SOS]                                # [E]
    x0T = np.zeros((128, 2 * BC), np.float32)
    for ec in range(2):
        x0T[:, ec * BC:(ec + 1) * BC] = np.repeat(
            x0[ec * 128:(ec + 1) * 128][:, None], BC, 1)
    wihK = w_ih.T.reshape(2, 128, 3 * H).transpose(1, 0, 2).reshape(128, 2 * 3 * H)
    whhK = w_hh.T.reshape(4, 128, 3 * H).transpose(1, 0, 2).reshape(128, 4 * 3 * H)
    Wah = Wa[:, 0:H]                             # [512g, 512h]
    WahK = Wah.T.reshape(4, 128, H).transpose(1, 0, 2).reshape(128, 4 * H)
    Wac = Wa[:, H:2 * H]
    eye8 = np.tile(np.eye(BC, dtype=np.float32).reshape(1, BC * BC), (128, 1))
    base_b = np.zeros((128, NBANK * 8), np.float32)
    for bank in range(NBANK):
        base_b[:, bank * 8:(bank + 1) * 8] = bank * 1024
    base_b[64:128, :] += 512

    in_maps = []
    for c in range(NC):
        bs = slice(c * BC, (c + 1) * BC)
        eo_c = eo[bs]                            # [BC, S, H]
        h0b = np.ascontiguousarray(h0[bs])
        h0T = h0b.T.reshape(4, 128, BC).transpose(1, 0, 2).reshape(128, 4 * BC)
        # epT[p, (gc*8+b)*64+s] = ep_c[b, s, gc*128+p], ep = eo @ attn_w.T
        ep_c = np.einsum("bsh,gh->bsg", eo_c, attn_w).astype(np.float32)
        epT = ep_c.transpose(2, 0, 1).reshape(4, 128, BC, S).transpose(
            1, 0, 2, 3).reshape(128, 4 * BC * S)
        eoW = np.einsum("bsh,gh->bsg", eo_c, Wac).astype(np.float32)
        eoWs = eoW.transpose(1, 0, 2).reshape(S, BC * H)
        fc_c = fc_w[c * VC:(c + 1) * VC]         # [VC, H]
        fcT = fc_c.T.reshape(4, 128, VC).transpose(1, 0, 2).reshape(128, 4 * VC)
        sel_m = np.zeros((B, BC), np.float32)
        for j in range(BC):
            sel_m[c * BC + j, j] = 1.0
        bofs = base_b + c * VC
        in_maps.append({
            "emb": emb, "x0T": x0T, "h0b": h0b, "h0T": h0T,
            "wihK": np.ascontiguousarray(wihK), "whhK": np.ascontiguousarray(whhK),
            "WahK": np.ascontiguousarray(WahK),
            "epT": np.ascontiguousarray(epT), "eoWs": np.ascontiguousarray(eoWs),
            "fcT": np.ascontiguousarray(fcT), "sel": sel_m,
            "eye8r": eye8, "bankofs": bofs.astype(np.float32),
        })
    return in_maps


def kernel(**inputs):
    if "nc" not in _CACHE:
        _CACHE["nc"] = build(T)
    nc = _CACHE["nc"]
    in_maps = _prep_inputs(inputs, T)
    res = bass_utils.run_bass_kernel_spmd(nc, in_maps, core_ids=list(range(NC)))
    outs = res.results
    dec = np.concatenate([outs[c]["o_logits"] for c in range(NC)], axis=2)
    h_fin = np.concatenate([outs[c]["o_h"] for c in range(NC)], axis=0)[None]
    return dec, h_fin


# revision 23
# speedup vs baseline: 1.2397x; 1.0342x over previous
"""Trainium2 Bass kernel for nn_Decoder (GRU + Luong attention + greedy decode).

Sharding: hybrid
  - GRU + attention: data-parallel over batch (8 rows/core)
  - fc ([V,H] @ h) + log-softmax stats + argmax: tensor-parallel over vocab
    (6250 rows/core, fc_w slice resident in SBUF)
  - two small AllGathers per step: concat_out ([8,512]->[64,512]) and
    per-shard (max, sumexp, argmax) stats ([128,4]->[1024,4])

Everything on the argmax feedback path is fp32 (top1-top2 logit gaps go down
to 3.5e-5 on these inputs; any bf16 would flip tokens and cascade).
"""

import numpy as np

import concourse.bacc as bacc
import concourse.bass as bass
import concourse.mybir as mybir
import concourse.tile as tile
from concourse.masks import make_identity
from concourse import bass_utils

F32 = mybir.dt.float32
I32 = mybir.dt.int32
U32 = mybir.dt.uint32
AF = mybir.ActivationFunctionType
OP = mybir.AluOpType
AX = mybir.AxisListType

NC = 8          # cores
B = 64          # batch
BC = B // NC    # batch rows per core
S = 64          # source length
E = 256         # embed dim
H = 512         # hidden
V = 50000       # vocab
VC = V // NC    # vocab rows per core
T = 32          # decode steps
SOS = 1

NTILE = 13          # vocab tiles per core: 12 x 512 + 1 x 106
TAIL_W = VC - 12 * 512   # 106
NBANK = 7           # fc psum passes (2 tiles/bank, bank 6 half-used)
STG_W = 6 * 512 + TAIL_W  # staged logits width per partition (3178)
BIG = 1.0e9

LN2 = 0.6931471805599453
# quadratic fit for log2(m), m in [1,2); refined by 2 Newton steps w/ Exp
LC2, LC1, LC0 = -0.344845, 2.024658, -1.674873


def _ap(t, off, dims):
    """Hand-built access pattern view: dims = [[stride, size], ...] (elements)."""
    tensor = t.tensor if isinstance(t, bass.AP) else t
    return bass.AP(tensor, off, dims)


def build(t_steps=T):
    nc = bacc.Bacc("TRN2", target_bir_lowering=False, debug=False, num_devices=NC)

    # ---------------- external inputs (per core) ----------------
    emb_d = nc.dram_tensor("emb", [V, E], F32, kind="ExternalInput")
    x0T_d = nc.dram_tensor("x0T", [128, 2 * BC], F32, kind="ExternalInput")
    h0b_d = nc.dram_tensor("h0b", [BC, H], F32, kind="ExternalInput")
    h0T_d = nc.dram_tensor("h0T", [128, 4 * BC], F32, kind="ExternalInput")
    wih_d = nc.dram_tensor("wihK", [128, 2 * 3 * H], F32, kind="ExternalInput")
    whh_d = nc.dram_tensor("whhK", [128, 4 * 3 * H], F32, kind="ExternalInput")
    WahK_d = nc.dram_tensor("WahK", [128, 4 * H], F32, kind="ExternalInput")
    epT_d = nc.dram_tensor("epT", [128, 4 * BC * S], F32, kind="ExternalInput")
    eoWs_d = nc.dram_tensor("eoWs", [S, BC * H], F32, kind="ExternalInput")
    fcT_d = nc.dram_tensor("fcT", [128, 4 * VC], F32, kind="ExternalInput")
    sel_d = nc.dram_tensor("sel", [B, BC], F32, kind="ExternalInput")
    eye_d = nc.dram_tensor("eye8r", [128, BC * BC], F32, kind="ExternalInput")
    bofs_d = nc.dram_tensor("bankofs", [128, NBANK * 8], F32, kind="ExternalInput")

    # ---------------- external outputs (per core) ----------------
    olog_d = nc.dram_tensor("o_logits", [B, t_steps, VC], F32, kind="ExternalOutput")
    oh_d = nc.dram_tensor("o_h", [BC, H], F32, kind="ExternalOutput")
    otok_d = nc.dram_tensor("o_tok", [B, t_steps], F32, kind="ExternalOutput")

    # ---------------- internal dram (collective bounce) ----------------
    cc1_in = nc.dram_tensor("cc1_in", [BC, H], F32)
    cc1_out = nc.dram_tensor("cc1_out", [B, H], F32, addr_space="Shared")
    cc2_in = nc.dram_tensor("cc2_in", [128, 4], F32)
    cc2_out = nc.dram_tensor("cc2_out", [128 * NC, 4], F32, addr_space="Shared")
    rgroups = [list(range(NC))]

    with tile.TileContext(nc) as tc:
        with tc.tile_pool(name="res", bufs=1) as res, \
             tc.tile_pool(name="state", bufs=1) as stp, \
             tc.tile_pool(name="work", bufs=1) as wk, \
             tc.tile_pool(name="stats", bufs=1) as st, \
             tc.tile_pool(name="stg", bufs=1) as stg, \
             tc.tile_pool(name="psRZ", bufs=1, space="PSUM") as psRZ, \
             tc.tile_pool(name="psCAT", bufs=2, space="PSUM") as psCAT, \
             tc.tile_pool(name="psS", bufs=2, space="PSUM") as psS, \
             tc.tile_pool(name="psFC", bufs=2, space="PSUM") as psFC:

            # ---------------- residents ----------------
            ident = res.tile([64, 64], F32)
            make_identity(nc, ident[:, :])
            wih = res.tile([128, 2 * 3 * H], F32)
            nc.sync.dma_start(wih[:, :], wih_d.ap())
            whh = res.tile([128, 4 * 3 * H], F32)
            nc.sync.dma_start(whh[:, :], whh_d.ap())
            WahK = res.tile([128, 4 * H], F32)
            nc.sync.dma_start(WahK[:, :], WahK_d.ap())
            epT = res.tile([128, 4 * BC * S], F32)
            nc.sync.dma_start(epT[:, :], epT_d.ap())
            eoWs = res.tile([S, BC * H], F32)
            nc.sync.dma_start(eoWs[:, :], eoWs_d.ap())
            fcT = res.tile([128, 4 * VC], F32)
            nc.sync.dma_start(fcT[:, :], fcT_d.ap())
            sel = res.tile([B, BC], F32)
            nc.sync.dma_start(sel[:, :], sel_d.ap())
            eye8 = res.tile([128, BC * BC], F32)
            nc.sync.dma_start(eye8[:, :], eye_d.ap())
            bankofs = res.tile([128, NBANK * 8], F32)
            nc.sync.dma_start(bankofs[:, :], bofs_d.ap())
            toks = res.tile([B, t_steps], F32)

            # initial state
            xT = stp.tile([128, 2 * BC], F32, tag="xT")
            nc.sync.dma_start(xT[:, :], x0T_d.ap())
            h_b = stp.tile([BC, H], F32, tag="h_b")
            nc.sync.dma_start(h_b[:, :], h0b_d.ap())
            hT = stp.tile([128, 4 * BC], F32, tag="hT")
            nc.sync.dma_start(hT[:, :], h0T_d.ap())

            ID8 = ident[0:BC, 0:BC]
            ID64 = ident[0:64, 0:64]


            def emit_gh(hT_s):
                rz_ps = psRZ.tile([BC, 1024], F32, space="PSUM", tag="rz")
                in_ps = psS.tile([BC, H], F32, space="PSUM", tag="s")
                hn_ps = psS.tile([BC, H], F32, space="PSUM", tag="s")
                for hc in range(4):
                    for nt in range(2):
                        sl = slice(nt * 512, nt * 512 + 512)
                        nc.tensor.matmul(
                            rz_ps[:, sl], hT_s[:, hc * BC:(hc + 1) * BC],
                            whh[:, hc * 1536 + nt * 512: hc * 1536 + nt * 512 + 512],
                            start=(hc == 0), stop=False)
                    nc.tensor.matmul(
                        hn_ps[:, :], hT_s[:, hc * BC:(hc + 1) * BC],
                        whh[:, hc * 1536 + 1024: hc * 1536 + 1536],
                        start=(hc == 0), stop=(hc == 3))
                return rz_ps, in_ps, hn_ps

            gh_tiles = emit_gh(hT)

            for t in range(t_steps):
                # ============ GRU ============
                rz_ps, in_ps, hn_ps = gh_tiles
                # gi contributions (gh was emitted at the end of the previous
                # step so it runs during the AG2/token window)
                for ec in range(2):
                    for nt in range(2):
                        sl = slice(nt * 512, nt * 512 + 512)
                        nc.tensor.matmul(
                            rz_ps[:, sl], xT[:, ec * BC:(ec + 1) * BC],
                            wih[:, ec * 1536 + nt * 512: ec * 1536 + nt * 512 + 512],
                            start=False, stop=(ec == 1))
                    nc.tensor.matmul(
                        in_ps[:, :], xT[:, ec * BC:(ec + 1) * BC],
                        wih[:, ec * 1536 + 1024: ec * 1536 + 1536],
                        start=(ec == 0), stop=(ec == 1))

                # gates: sigmoid(x) = 0.5*tanh(0.5x)+0.5 (stay in exp/tanh table set)
                rz = wk.tile([BC, 1024], F32, tag="rz_sb")
                nc.scalar.activation(rz[:, :], rz_ps[:, :], AF.Tanh, scale=0.5)
                nc.vector.tensor_scalar(out=rz[:, :], in0=rz[:, :], scalar1=0.5,
                                        scalar2=0.5, op0=OP.mult, op1=OP.add)
                t1 = wk.tile([BC, H], F32, tag="t1")
                nc.vector.tensor_tensor(out=t1[:, :], in0=rz[:, 0:512], in1=hn_ps[:, :], op=OP.mult)
                nc.vector.tensor_tensor(out=t1[:, :], in0=t1[:, :], in1=in_ps[:, :], op=OP.add)
                nc.scalar.activation(t1[:, :], t1[:, :], AF.Tanh)   # t1 = n
                d_t = wk.tile([BC, H], F32, tag="d_t")
                nc.vector.tensor_tensor(out=d_t[:, :], in0=h_b[:, :], in1=t1[:, :], op=OP.subtract)
                nc.vector.tensor_tensor(out=d_t[:, :], in0=rz[:, 512:1024], in1=d_t[:, :], op=OP.mult)
                h_new = stp.tile([BC, H], F32, tag="h_b")
                nc.vector.tensor_tensor(out=h_new[:, :], in0=t1[:, :], in1=d_t[:, :], op=OP.add)
                h_b = h_new

                # hT update: 4 PE transposes [8,128] -> [128,8]
                tps = psS.tile([128, 4 * BC], F32, space="PSUM", tag="s")
                for hc in range(4):
                    nc.tensor.transpose(tps[:, hc * BC:(hc + 1) * BC],
                                        h_new[:, hc * 128:(hc + 1) * 128], ID8)
                hT_new = stp.tile([128, 4 * BC], F32, tag="hT")
                nc.vector.tensor_copy(hT_new[:, :], tps[:, :])
                hT = hT_new

                # ============ attention ============
                # cross[b,(bp,s)] = sum_h hT[h,b] * enc_proj[bp][h,s]; 4 K-chunks
                cr_h = [psS.tile([BC, 256], F32, space="PSUM", tag="s",
                                 name=f"crh{q}_{t}")
                        for q in range(2)]
                for hc in range(4):
                    for q in range(2):
                        nc.tensor.matmul(
                            cr_h[q][:, :],
                            hT[:, hc * BC:(hc + 1) * BC],
                            epT[:, hc * BC * S + q * 256: hc * BC * S + q * 256 + 256],
                            start=(hc == 0), stop=(hc == 3))
                # mask to the diagonal block and reduce over bp
                crm = wk.tile([BC, BC * S], F32, tag="crm")
                for q in range(2):
                    nc.vector.tensor_tensor(
                        out=_ap(crm, q * 4 * S, [[BC * S, BC], [S, 4], [1, S]]),
                        in0=_ap(cr_h[q], 0, [[256, BC], [S, 4], [1, S]]),
                        in1=_ap(ident, q * 4, [[64, BC], [1, 4], [0, S]]),
                        op=OP.mult)
                sc_sb = wk.tile([BC, S], F32, tag="sc_sb")
                nc.vector.tensor_reduce(
                    _ap(sc_sb, 0, [[S, BC], [1, S]]),
                    _ap(crm, 0, [[BC * S, BC], [1, S], [S, BC]]),
                    axis=AX.X, op=OP.add)

                # softmax over s
                nmx = wk.tile([BC, 1], F32, tag="nmx")
                nc.vector.reduce_max(nmx[:, :], sc_sb[:, :], axis=AX.X)
                nc.vector.tensor_scalar_mul(nmx[:, :], nmx[:, :], -1.0)
                aw = wk.tile([BC, S], F32, tag="aw")
                sume = wk.tile([BC, 1], F32, tag="sume")
                nc.scalar.activation(aw[:, :], sc_sb[:, :], AF.Exp,
                                     bias=nmx[:, :], scale=1.0, accum_out=sume[:, :])
                rec = wk.tile([BC, 1], F32, tag="rec")
                nc.vector.reciprocal(rec[:, :], sume[:, :])
                nc.vector.tensor_scalar(out=aw[:, :], in0=aw[:, :], scalar1=rec[:, :],
                                        scalar2=None, op0=OP.mult)

                # awT + mask
                awT_ps = psS.tile([S, BC], F32, space="PSUM", tag="s")
                nc.tensor.transpose(awT_ps[:, :], aw[:, :], ID8)
                awT = wk.tile([S, BC], F32, tag="awT")
                nc.vector.tensor_copy(awT[:, :], awT_ps[:, :])
                awm = wk.tile([S, BC * BC], F32, tag="awm")
                nc.vector.tensor_tensor(
                    out=_ap(awm, 0, [[BC * BC, S], [BC, BC], [1, BC]]),
                    in0=_ap(awT, 0, [[BC, S], [1, BC], [0, BC]]),
                    in1=_ap(eye8, 0, [[BC * BC, S], [BC, BC], [1, BC]]),
                    op=OP.mult)

                # concat_out = tanh(ctx @ Wa_c.T + h @ Wa_h.T)
                cat_h = [psCAT.tile([BC, 256], F32, space="PSUM", tag="catps",
                                    name=f"cath{q}_{t}")
                         for q in range(2)]
                for bp in range(BC):
                    lhsT = _ap(awm, bp, [[BC * BC, S], [BC, BC]])
                    for q in range(2):
                        nc.tensor.matmul(cat_h[q][:, :], lhsT,
                                         eoWs[0:S, bp * H + q * 256: bp * H + q * 256 + 256],
                                         start=(bp == 0), stop=False)
                for hc in range(4):
                    for q in range(2):
                        nc.tensor.matmul(cat_h[q][:, :],
                                         hT[:, hc * BC:(hc + 1) * BC],
                                         WahK[:, hc * H + q * 256: hc * H + q * 256 + 256],
                                         start=False, stop=(hc == 3))
                co = wk.tile([BC, H], F32, tag="co")
                for q in range(2):
                    nc.scalar.activation(co[:, q * 256:(q + 1) * 256],
                                         cat_h[q][:, :], AF.Tanh)

                # ============ AllGather #1: concat_out ============
                nc.sync.dma_start(cc1_in.ap(), co[:, :])
                nc.gpsimd.collective_compute(
                    "AllGather", OP.bypass, replica_groups=rgroups,
                    ins=[cc1_in.ap().opt()], outs=[cc1_out.ap().opt()])
                coall = wk.tile([B, H], F32, tag="coall")
                nc.sync.dma_start(coall[:, :], cc1_out.ap())

                # concatT: [64,512] -> [128, 4*64]
                cT_ps = psS.tile([128, 4 * 64], F32, space="PSUM", tag="s")
                for hc in range(4):
                    nc.tensor.transpose(cT_ps[:, hc * 64:(hc + 1) * 64],
                                        coall[:, hc * 128:(hc + 1) * 128], ID64)
                cT = wk.tile([128, 4 * 64], F32, tag="cT")
                nc.vector.tensor_copy(cT[:, :], cT_ps[:, :])

                # ============ fc + per-bank stats ============
                staged = stg.tile([128, STG_W], F32, tag="staged")
                bmax8 = st.tile([128, NBANK * 8], F32, tag="bmax8")
                bidx8 = st.tile([128, NBANK * 8], U32, tag="bidx8")
                sume_b = st.tile([128, NBANK], F32, tag="sume_b")
                nc.vector.memset(bmax8[64:128, 48:56], -BIG)
                nc.vector.memset(bidx8[64:128, 48:56], 0)
                nc.vector.memset(sume_b[64:128, 6:7], 0.0)

                for bank in range(NBANK):
                    fc_ps = psFC.tile([128, 512], F32, space="PSUM", tag="fcps")
                    for kc in range(4):
                        for hf in range(2):
                            tt_i = 2 * bank + hf
                            if tt_i >= NTILE:
                                continue
                            w = 512 if tt_i < 12 else TAIL_W
                            pr = fc_ps[64 * hf:64 * hf + 64, 0:w]
                            nc.tensor.matmul(
                                pr,
                                cT[:, kc * 64:(kc + 1) * 64],
                                fcT[:, kc * VC + tt_i * 512: kc * VC + tt_i * 512 + w],
                                start=(kc == 0), stop=(kc == 3),
                                tile_position=(0, 64) if hf else None,
                                skip_group_check=True)
                    pmax = 128 if bank < 6 else 64
                    w_eff = 512 if bank < 6 else TAIL_W
                    s_sl = staged[0:pmax, bank * 512: bank * 512 + w_eff]
                    nc.scalar.copy(s_sl, fc_ps[0:pmax, 0:w_eff])
                    # exp in place on the psum bank (raw values already staged)
                    nc.scalar.activation(fc_ps[0:pmax, 0:w_eff], fc_ps[0:pmax, 0:w_eff],
                                         AF.Exp, accum_out=sume_b[0:pmax, bank:bank + 1])
                    nc.vector.max(out=bmax8[0:pmax, bank * 8:(bank + 1) * 8], in_=s_sl)
                    nc.vector.max_index(out=bidx8[0:pmax, bank * 8:(bank + 1) * 8],
                                        in_max=bmax8[0:pmax, bank * 8:(bank + 1) * 8],
                                        in_values=s_sl)

                # next step's gh matmuls: only need hT, so they fill the
                # PE pipe during the stats/AG2/token window
                if t < t_steps - 1:
                    gh_tiles = emit_gh(hT)

                # ============ local stats combine ============
                stats = st.tile([128, 4], F32, tag="stats")
                nc.vector.reduce_max(stats[:, 0:1], bmax8[:, :], axis=AX.X)
                nc.vector.reduce_sum(stats[:, 1:2], sume_b[:, :], axis=AX.X)
                cidx = st.tile([128, NBANK * 8], F32, tag="cidx")
                nc.vector.tensor_copy(cidx[:, :], bidx8[:, :])
                nc.vector.tensor_tensor(out=cidx[:, :], in0=cidx[:, :], in1=bankofs[:, :], op=OP.add)
                eqm = st.tile([128, NBANK * 8], F32, tag="eqm")
                nc.vector.tensor_tensor(out=eqm[:, :], in0=bmax8[:, :],
                                        in1=_ap(stats, 0, [[4, 128], [0, NBANK * 8]]),
                                        op=OP.is_equal)
                nc.vector.tensor_scalar(out=eqm[:, :], in0=eqm[:, :], scalar1=-BIG,
                                        scalar2=BIG, op0=OP.mult, op1=OP.add)
                nc.vector.tensor_tensor(out=cidx[:, :], in0=cidx[:, :], in1=eqm[:, :], op=OP.add)
                nc.vector.tensor_reduce(stats[:, 2:3], cidx[:, :], axis=AX.X, op=OP.min)
                nc.vector.memset(stats[:, 3:4], 0.0)

                # ============ AllGather #2: stats ============
                nc.sync.dma_start(cc2_in.ap(), stats[:, :])
                nc.gpsimd.collective_compute(
                    "AllGather", OP.bypass, replica_groups=rgroups,
                    ins=[cc2_in.ap().opt()], outs=[cc2_out.ap().opt()])
                gath = st.tile([128, 16 * 4], F32, tag="gath")
                gsrc = _ap(cc2_out, 0, [[4, 64], [512, NC], [256, 2], [1, 4]])
                nc.sync.dma_start(gath[0:64, :], gsrc)

                # global combine: token path first, on partitions 0:64 only
                lmaxs64 = _ap(gath, 0, [[64, 64], [4, 16]])
                gmax = st.tile([128, 1], F32, tag="gmax")
                nc.vector.reduce_max(gmax[0:64, :], lmaxs64, axis=AX.X)

                if t < t_steps - 1:
                    eq2 = st.tile([64, 16], F32, tag="eq2")
                    nc.vector.tensor_tensor(out=eq2[:, :], in0=lmaxs64,
                                            in1=_ap(gmax, 0, [[1, 64], [0, 16]]),
                                            op=OP.is_equal)
                    nc.vector.tensor_scalar(out=eq2[:, :], in0=eq2[:, :], scalar1=-BIG,
                                            scalar2=BIG, op0=OP.mult, op1=OP.add)
                    nc.vector.tensor_tensor(out=eq2[:, :], in0=eq2[:, :],
                                            in1=_ap(gath, 2, [[64, 64], [4, 16]]), op=OP.add)
                    tokf = st.tile([64, 1], F32, tag="tokf")
                    nc.vector.tensor_reduce(tokf[:, :], eq2[:, :], axis=AX.X, op=OP.min)
                    nc.vector.tensor_copy(toks[:, t + 1:t + 2], tokf[0:B, :])

                    # my 8 tokens -> indices -> embedding gather -> xT
                    tok_ps = psS.tile([BC, 1], F32, space="PSUM", tag="s")
                    nc.tensor.matmul(tok_ps[:, :], sel[:, :], tokf[:, :],
                                     start=True, stop=True)
                    tok_i = wk.tile([BC, 1], I32, tag="toki")
                    nc.vector.tensor_copy(tok_i[:, :], tok_ps[:, :])
                    x_g = wk.tile([BC, E], F32, tag="xg")
                    nc.gpsimd.indirect_dma_start(
                        out=x_g[:, :], out_offset=None, in_=emb_d.ap(),
                        in_offset=bass.IndirectOffsetOnAxis(ap=tok_i[:, 0:1], axis=0))
                    xt_ps = psS.tile([128, 2 * BC], F32, space="PSUM", tag="s")
                    for ec in range(2):
                        nc.tensor.transpose(xt_ps[:, ec * BC:(ec + 1) * BC],
                                            x_g[:, ec * 128:(ec + 1) * 128], ID8)
                    xT_new = stp.tile([128, 2 * BC], F32, tag="xT")
                    nc.vector.tensor_copy(xT_new[:, :], xt_ps[:, :])
                    xT = xT_new

                # per-shard sumexps were computed without max subtraction
                # (logits are O(1)), so lse = ln(sum_j sume_j) directly
                nc.sync.dma_start(gath[64:128, :], gsrc)
                ssum = st.tile([128, 1], F32, tag="ssum")
                nc.vector.reduce_sum(ssum[:, :], _ap(gath, 1, [[64, 128], [4, 16]]),
                                     axis=AX.X)

                # ln(ssum) via exponent bits + quadratic + 2 Newton steps (Exp only)
                si = ssum[:, :].bitcast(I32)
                e_i = st.tile([128, 1], I32, tag="e_i")
                nc.vector.tensor_scalar(out=e_i[:, :], in0=si, scalar1=23,
                                        scalar2=None, op0=OP.arith_shift_right)
                e_f = st.tile([128, 1], F32, tag="e_f")
                nc.vector.tensor_copy(e_f[:, :], e_i[:, :])
                nc.vector.tensor_scalar(out=e_f[:, :], in0=e_f[:, :], scalar1=-127.0,
                                        scalar2=None, op0=OP.add)
                m_i = st.tile([128, 1], I32, tag="m_i")
                nc.vector.tensor_scalar(out=m_i[:, :], in0=si, scalar1=0x7FFFFF,
                                        scalar2=None, op0=OP.bitwise_and)
                nc.vector.tensor_scalar(out=m_i[:, :], in0=m_i[:, :], scalar1=0x3F800000,
                                        scalar2=None, op0=OP.bitwise_or)
                m_f = m_i[:, :].bitcast(F32)
                poly = st.tile([128, 1], F32, tag="poly")
                nc.vector.tensor_scalar(out=poly[:, :], in0=m_f, scalar1=LC2,
                                        scalar2=LC1, op0=OP.mult, op1=OP.add)
                nc.vector.tensor_tensor(out=poly[:, :], in0=poly[:, :], in1=m_f, op=OP.mult)
                nc.vector.tensor_scalar(out=poly[:, :], in0=poly[:, :], scalar1=LC0,
                                        scalar2=None, op0=OP.add)
                lnv = st.tile([128, 1], F32, tag="lnv")
                nc.vector.tensor_tensor(out=lnv[:, :], in0=poly[:, :], in1=e_f[:, :], op=OP.add)
                nc.vector.tensor_scalar(out=lnv[:, :], in0=lnv[:, :], scalar1=LN2,
                                        scalar2=None, op0=OP.mult)
                for _ in range(2):
                    nx = st.tile([128, 1], F32, tag="nx")
                    nc.scalar.activation(nx[:, :], lnv[:, :], AF.Exp, scale=-1.0)
                    nc.vector.tensor_tensor(out=nx[:, :], in0=nx[:, :], in1=ssum[:, :], op=OP.mult)
                    nc.vector.tensor_scalar(out=nx[:, :], in0=nx[:, :], scalar1=-1.0,
                                            scalar2=None, op0=OP.add)
                    nc.vector.tensor_tensor(out=lnv[:, :], in0=lnv[:, :], in1=nx[:, :], op=OP.add)

                nlse = st.tile([128, 1], F32, tag="nlse")
                nc.vector.tensor_scalar_mul(nlse[:, :], lnv[:, :], -1.0)

                # subtract lse in place, then write out
                nc.vector.tensor_scalar(out=staged[:, 0:3072], in0=staged[:, 0:3072],
                                        scalar1=nlse[:, :], scalar2=None, op0=OP.add)
                nc.vector.tensor_scalar(out=staged[0:64, 3072:STG_W],
                                        in0=staged[0:64, 3072:STG_W],
                                        scalar1=nlse[0:64, :], scalar2=None, op0=OP.add)
                dst0 = _ap(olog_d, t * VC, [[t_steps * VC, B], [1024, 6], [1, 512]])
                nc.sync.dma_start(dst0, staged[0:64, 0:3072].rearrange("p (a b) -> p a b", a=6))
                dst1 = _ap(olog_d, t * VC + 512, [[t_steps * VC, B], [1024, 6], [1, 512]])
                nc.sync.dma_start(dst1, staged[64:128, 0:3072].rearrange("p (a b) -> p a b", a=6))
                dst2 = _ap(olog_d, t * VC + 6144, [[t_steps * VC, B], [1, TAIL_W]])
                nc.sync.dma_start(dst2, staged[0:64, 3072:3072 + TAIL_W])

            # epilogue
            nc.sync.dma_start(oh_d.ap(), h_b[:, :])
            nc.vector.memset(toks[:, 0:1], float(SOS))
            nc.sync.dma_start(otok_d.ap(), toks[:, :])

    nc.compile()
    return nc


# ======================= host side =======================

_CACHE = {}


def _prep_inputs(inputs, t_steps=T):
    emb = np.ascontiguousarray(np.asarray(inputs["embedding"], dtype=np.float32))
    eh = np.asarray(inputs["encoder_hidden"], dtype=np.float32)
    eo = np.asarray(inputs["encoder_outputs"], dtype=np.float32)
    w_ih = np.asarray(inputs["w_ih"], dtype=np.float32)
    w_hh = np.asarray(inputs["w_hh"], dtype=np.float32)
    attn_w = np.asarray(inputs["attn_w"], dtype=np.float32)
    Wa = np.asarray(inputs["Wa"], dtype=np.float32)
    fc_w = np.asarray(inputs["fc_w"], dtype=np.float32)

    h0 = eh[0]                                   # [B, H]
    x0 = emb[SOS]                                # [E]
    x0T = np.zeros((128, 2 * BC), np.float32)
    for ec in range(2):
        x0T[:, ec * BC:(ec + 1) * BC] = np.repeat(
            x0[ec * 128:(ec + 1) * 128][:, None], BC, 1)
    wihK = w_ih.T.reshape(2, 128, 3 * H).transpose(1, 0, 2).reshape(128, 2 * 3 * H)
    whhK = w_hh.T.reshape(4, 128, 3 * H).transpose(1, 0, 2).reshape(128, 4 * 3 * H)
    Wah = Wa[:, 0:H]                             # [512g, 512h]
    WahK = Wah.T.reshape(4, 128, H).transpose(1, 0, 2).reshape(128, 4 * H)
    Wac = Wa[:, H:2 * H]
    eye8 = np.tile(np.eye(BC, dtype=np.float32).reshape(1, BC * BC), (128, 1))
    base_b = np.zeros((128, NBANK * 8), np.float32)
    for bank in range(NBANK):
        base_b[:, bank * 8:(bank + 1) * 8] = bank * 1024
    base_b[64:128, :] += 512

    in_maps = []
    for c in range(NC):
        bs = slice(c * BC, (c + 1) * BC)
        eo_c = eo[bs]                            # [BC, S, H]
        h0b = np.ascontiguousarray(h0[bs])
        h0T = h0b.T.reshape(4, 128, BC).transpose(1, 0, 2).reshape(128, 4 * BC)
        # epT[p, (gc*8+b)*64+s] = ep_c[b, s, gc*128+p], ep = eo @ attn_w.T
        ep_c = np.einsum("bsh,gh->bsg", eo_c, attn_w).astype(np.float32)
        epT = ep_c.transpose(2, 0, 1).reshape(4, 128, BC, S).transpose(
            1, 0, 2, 3).reshape(128, 4 * BC * S)
        eoW = np.einsum("bsh,gh->bsg", eo_c, Wac).astype(np.float32)
        eoWs = eoW.transpose(1, 0, 2).reshape(S, BC * H)
        fc_c = fc_w[c * VC:(c + 1) * VC]         # [VC, H]
        fcT = fc_c.T.reshape(4, 128, VC).transpose(1, 0, 2).reshape(128, 4 * VC)
        sel_m = np.zeros((B, BC), np.float32)
        for j in range(BC):
            sel_m[c * BC + j, j] = 1.0
        bofs = base_b + c * VC
        in_maps.append({
            "emb": emb, "x0T": x0T, "h0b": h0b, "h0T": h0T,
            "wihK": np.ascontiguousarray(wihK), "whhK": np.ascontiguousarray(whhK),
            "WahK": np.ascontiguousarray(WahK),
            "epT": np.ascontiguousarray(epT), "eoWs": np.ascontiguousarray(eoWs),
            "fcT": np.ascontiguousarray(fcT), "sel": sel_m,
            "eye8r": eye8, "bankofs": bofs.astype(np.float32),
        })
    return in_maps


def kernel(**inputs):
    if "nc" not in _CACHE:
        _CACHE["nc"] = build(T)
    nc = _CACHE["nc"]
    in_maps = _prep_inputs(inputs, T)
    res = bass_utils.run_bass_kernel_spmd(nc, in_maps, core_ids=list(range(NC)))
    outs = res.results
    dec = np.concatenate([outs[c]["o_logits"] for c in range(NC)], axis=2)
    h_fin = np.concatenate([outs[c]["o_h"] for c in range(NC)], axis=0)[None]
    return dec, h_fin


# revision 26
# speedup vs baseline: 1.3062x; 1.0536x over previous
"""Trainium2 Bass kernel for nn_Decoder (GRU + Luong attention + greedy decode).

Sharding: hybrid
  - GRU + attention: data-parallel over batch (8 rows/core)
  - fc ([V,H] @ h) + log-softmax stats + argmax: tensor-parallel over vocab
    (6250 rows/core, fc_w slice resident in SBUF)
  - two small AllGathers per step: concat_out ([8,512]->[64,512]) and
    per-shard (max, sumexp, argmax) stats ([128,4]->[1024,4])

Everything on the argmax feedback path is fp32 (top1-top2 logit gaps go down
to 3.5e-5 on these inputs; any bf16 would flip tokens and cascade).
"""

import numpy as np

import concourse.bacc as bacc
import concourse.bass as bass
import concourse.mybir as mybir
import concourse.tile as tile
from concourse.masks import make_identity
from concourse import bass_utils

F32 = mybir.dt.float32
I32 = mybir.dt.int32
U32 = mybir.dt.uint32
AF = mybir.ActivationFunctionType
OP = mybir.AluOpType
AX = mybir.AxisListType

NC = 8          # cores
B = 64          # batch
BC = B // NC    # batch rows per core
S = 64          # source length
E = 256         # embed dim
H = 512         # hidden
V = 50000       # vocab
VC = V // NC    # vocab rows per core
T = 32          # decode steps
SOS = 1

NTILE = 13          # vocab tiles per core: 12 x 512 + 1 x 106
TAIL_W = VC - 12 * 512   # 106
NBANK = 7           # fc psum passes (2 tiles/bank, bank 6 half-used)
STG_W = 6 * 512 + TAIL_W  # staged logits width per partition (3178)
BIG = 1.0e9

LN2 = 0.6931471805599453
# quadratic fit for log2(m), m in [1,2); refined by 2 Newton steps w/ Exp
LC2, LC1, LC0 = -0.344845, 2.024658, -1.674873


def _ap(t, off, dims):
    """Hand-built access pattern view: dims = [[stride, size], ...] (elements)."""
    tensor = t.tensor if isinstance(t, bass.AP) else t
    return bass.AP(tensor, off, dims)


def build(t_steps=T):
    nc = bacc.Bacc("TRN2", target_bir_lowering=False, debug=False, num_devices=NC)

    # ---------------- external inputs (per core) ----------------
    emb_d = nc.dram_tensor("emb", [V, E], F32, kind="ExternalInput")
    x0T_d = nc.dram_tensor("x0T", [128, 2 * BC], F32, kind="ExternalInput")
    h0b_d = nc.dram_tensor("h0b", [BC, H], F32, kind="ExternalInput")
    h0T_d = nc.dram_tensor("h0T", [128, 4 * BC], F32, kind="ExternalInput")
    wih_d = nc.dram_tensor("wihK", [128, 2 * 3 * H], F32, kind="ExternalInput")
    whh_d = nc.dram_tensor("whhK", [128, 4 * 3 * H], F32, kind="ExternalInput")
    WahK_d = nc.dram_tensor("WahK", [128, 4 * H], F32, kind="ExternalInput")
    epT_d = nc.dram_tensor("epT", [128, 4 * BC * S], F32, kind="ExternalInput")
    eoWs_d = nc.dram_tensor("eoWs", [S, BC * H], F32, kind="ExternalInput")
    fcT_d = nc.dram_tensor("fcT", [128, 4 * VC], F32, kind="ExternalInput")
    sel_d = nc.dram_tensor("sel", [B, BC], F32, kind="ExternalInput")
    eye_d = nc.dram_tensor("eye8r", [128, BC * BC], F32, kind="ExternalInput")
    bofs_d = nc.dram_tensor("bankofs", [128, NBANK * 8], F32, kind="ExternalInput")

    # ---------------- external outputs (per core) ----------------
    olog_d = nc.dram_tensor("o_logits", [B, t_steps, VC], F32, kind="ExternalOutput")
    oh_d = nc.dram_tensor("o_h", [BC, H], F32, kind="ExternalOutput")
    otok_d = nc.dram_tensor("o_tok", [B, t_steps], F32, kind="ExternalOutput")

    # ---------------- internal dram (collective bounce) ----------------
    cc1_in = nc.dram_tensor("cc1_in", [BC, H], F32)
    cc1_out = nc.dram_tensor("cc1_out", [B, H], F32, addr_space="Shared")
    cc2_in = nc.dram_tensor("cc2_in", [128, 4], F32)
    cc2_out = nc.dram_tensor("cc2_out", [128 * NC, 4], F32, addr_space="Shared")
    rgroups = [list(range(NC))]

    with tile.TileContext(nc) as tc:
        with tc.tile_pool(name="res", bufs=1) as res, \
             tc.tile_pool(name="state", bufs=1) as stp, \
             tc.tile_pool(name="work", bufs=1) as wk, \
             tc.tile_pool(name="stats", bufs=1) as st, \
             tc.tile_pool(name="stg", bufs=1) as stg, \
             tc.tile_pool(name="psRZ", bufs=1, space="PSUM") as psRZ, \
             tc.tile_pool(name="psCAT", bufs=2, space="PSUM") as psCAT, \
             tc.tile_pool(name="psS", bufs=2, space="PSUM") as psS, \
             tc.tile_pool(name="psFC", bufs=2, space="PSUM") as psFC:

            # ---------------- residents ----------------
            ident = res.tile([64, 64], F32)
            make_identity(nc, ident[:, :])
            wih = res.tile([128, 2 * 3 * H], F32)
            nc.sync.dma_start(wih[:, :], wih_d.ap())
            whh = res.tile([128, 4 * 3 * H], F32)
            nc.sync.dma_start(whh[:, :], whh_d.ap())
            WahK = res.tile([128, 4 * H], F32)
            nc.sync.dma_start(WahK[:, :], WahK_d.ap())
            epT = res.tile([128, 4 * BC * S], F32)
            nc.sync.dma_start(epT[:, :], epT_d.ap())
            eoWs = res.tile([S, BC * H], F32)
            nc.sync.dma_start(eoWs[:, :], eoWs_d.ap())
            fcT = res.tile([128, 4 * VC], F32)
            nc.sync.dma_start(fcT[:, :], fcT_d.ap())
            sel = res.tile([B, BC], F32)
            nc.sync.dma_start(sel[:, :], sel_d.ap())
            eye8 = res.tile([128, BC * BC], F32)
            nc.sync.dma_start(eye8[:, :], eye_d.ap())
            bankofs = res.tile([128, NBANK * 8], F32)
            nc.sync.dma_start(bankofs[:, :], bofs_d.ap())
            toks = res.tile([B, t_steps], F32)

            # initial state
            xT = stp.tile([128, 2 * BC], F32, tag="xT")
            nc.sync.dma_start(xT[:, :], x0T_d.ap())
            h_b = stp.tile([BC, H], F32, tag="h_b")
            nc.sync.dma_start(h_b[:, :], h0b_d.ap())
            hT = stp.tile([128, 4 * BC], F32, tag="hT")
            nc.sync.dma_start(hT[:, :], h0T_d.ap())

            ID8 = ident[0:BC, 0:BC]
            ID64 = ident[0:64, 0:64]


            def emit_gh(hT_s):
                rz_ps = psRZ.tile([BC, 1024], F32, space="PSUM", tag="rz")
                in_ps = psS.tile([BC, H], F32, space="PSUM", tag="s")
                hn_ps = psS.tile([BC, H], F32, space="PSUM", tag="s")
                for hc in range(4):
                    for nt in range(2):
                        sl = slice(nt * 512, nt * 512 + 512)
                        nc.tensor.matmul(
                            rz_ps[:, sl], hT_s[:, hc * BC:(hc + 1) * BC],
                            whh[:, hc * 1536 + nt * 512: hc * 1536 + nt * 512 + 512],
                            start=(hc == 0), stop=False)
                    nc.tensor.matmul(
                        hn_ps[:, :], hT_s[:, hc * BC:(hc + 1) * BC],
                        whh[:, hc * 1536 + 1024: hc * 1536 + 1536],
                        start=(hc == 0), stop=(hc == 3))
                return rz_ps, in_ps, hn_ps

            gh_tiles = emit_gh(hT)

            for t in range(t_steps):
                # ============ GRU ============
                rz_ps, in_ps, hn_ps = gh_tiles
                # gi contributions (gh was emitted at the end of the previous
                # step so it runs during the AG2/token window)
                for ec in range(2):
                    for nt in range(2):
                        sl = slice(nt * 512, nt * 512 + 512)
                        nc.tensor.matmul(
                            rz_ps[:, sl], xT[:, ec * BC:(ec + 1) * BC],
                            wih[:, ec * 1536 + nt * 512: ec * 1536 + nt * 512 + 512],
                            start=False, stop=(ec == 1))
                    nc.tensor.matmul(
                        in_ps[:, :], xT[:, ec * BC:(ec + 1) * BC],
                        wih[:, ec * 1536 + 1024: ec * 1536 + 1536],
                        start=(ec == 0), stop=(ec == 1))

                # gates: sigmoid(x) = 0.5*tanh(0.5x)+0.5 (stay in exp/tanh table set)
                rz = wk.tile([BC, 1024], F32, tag="rz_sb")
                nc.scalar.activation(rz[:, :], rz_ps[:, :], AF.Tanh, scale=0.5)
                nc.vector.tensor_scalar(out=rz[:, :], in0=rz[:, :], scalar1=0.5,
                                        scalar2=0.5, op0=OP.mult, op1=OP.add)
                t1 = wk.tile([BC, H], F32, tag="t1")
                nc.vector.tensor_tensor(out=t1[:, :], in0=rz[:, 0:512], in1=hn_ps[:, :], op=OP.mult)
                nc.vector.tensor_tensor(out=t1[:, :], in0=t1[:, :], in1=in_ps[:, :], op=OP.add)
                nc.scalar.activation(t1[:, :], t1[:, :], AF.Tanh)   # t1 = n
                d_t = wk.tile([BC, H], F32, tag="d_t")
                nc.vector.tensor_tensor(out=d_t[:, :], in0=h_b[:, :], in1=t1[:, :], op=OP.subtract)
                nc.vector.tensor_tensor(out=d_t[:, :], in0=rz[:, 512:1024], in1=d_t[:, :], op=OP.mult)
                h_new = stp.tile([BC, H], F32, tag="h_b")
                nc.vector.tensor_tensor(out=h_new[:, :], in0=t1[:, :], in1=d_t[:, :], op=OP.add)
                h_b = h_new

                # hT update: 4 PE transposes [8,128] -> [128,8]
                tps = psS.tile([128, 4 * BC], F32, space="PSUM", tag="s")
                for hc in range(4):
                    nc.tensor.transpose(tps[:, hc * BC:(hc + 1) * BC],
                                        h_new[:, hc * 128:(hc + 1) * 128], ID8)
                hT_new = stp.tile([128, 4 * BC], F32, tag="hT")
                nc.vector.tensor_copy(hT_new[:, :], tps[:, :])
                hT = hT_new

                # ============ attention ============
                # cross[b,(bp,s)] = sum_h hT[h,b] * enc_proj[bp][h,s]; 4 K-chunks
                cr_h = [psS.tile([BC, 256], F32, space="PSUM", tag="s",
                                 name=f"crh{q}_{t}")
                        for q in range(2)]
                for hc in range(4):
                    for q in range(2):
                        nc.tensor.matmul(
                            cr_h[q][:, :],
                            hT[:, hc * BC:(hc + 1) * BC],
                            epT[:, hc * BC * S + q * 256: hc * BC * S + q * 256 + 256],
                            start=(hc == 0), stop=(hc == 3))
                # mask to the diagonal block and reduce over bp
                crm = wk.tile([BC, BC * S], F32, tag="crm")
                for q in range(2):
                    nc.vector.tensor_tensor(
                        out=_ap(crm, q * 4 * S, [[BC * S, BC], [S, 4], [1, S]]),
                        in0=_ap(cr_h[q], 0, [[256, BC], [S, 4], [1, S]]),
                        in1=_ap(ident, q * 4, [[64, BC], [1, 4], [0, S]]),
                        op=OP.mult)
                sc_sb = wk.tile([BC, S], F32, tag="sc_sb")
                nc.vector.tensor_reduce(
                    _ap(sc_sb, 0, [[S, BC], [1, S]]),
                    _ap(crm, 0, [[BC * S, BC], [1, S], [S, BC]]),
                    axis=AX.X, op=OP.add)

                # softmax over s
                nmx = wk.tile([BC, 1], F32, tag="nmx")
                nc.vector.reduce_max(nmx[:, :], sc_sb[:, :], axis=AX.X)
                nc.vector.tensor_scalar_mul(nmx[:, :], nmx[:, :], -1.0)
                aw = wk.tile([BC, S], F32, tag="aw")
                sume = wk.tile([BC, 1], F32, tag="sume")
                nc.scalar.activation(aw[:, :], sc_sb[:, :], AF.Exp,
                                     bias=nmx[:, :], scale=1.0, accum_out=sume[:, :])
                rec = wk.tile([BC, 1], F32, tag="rec")
                nc.vector.reciprocal(rec[:, :], sume[:, :])
                nc.vector.tensor_scalar(out=aw[:, :], in0=aw[:, :], scalar1=rec[:, :],
                                        scalar2=None, op0=OP.mult)

                # awT + mask
                awT_ps = psS.tile([S, BC], F32, space="PSUM", tag="s")
                nc.tensor.transpose(awT_ps[:, :], aw[:, :], ID8)
                awT = wk.tile([S, BC], F32, tag="awT")
                nc.vector.tensor_copy(awT[:, :], awT_ps[:, :])
                awm = wk.tile([S, BC * BC], F32, tag="awm")
                nc.vector.tensor_tensor(
                    out=_ap(awm, 0, [[BC * BC, S], [BC, BC], [1, BC]]),
                    in0=_ap(awT, 0, [[BC, S], [1, BC], [0, BC]]),
                    in1=_ap(eye8, 0, [[BC * BC, S], [BC, BC], [1, BC]]),
                    op=OP.mult)

                # concat_out = tanh(ctx @ Wa_c.T + h @ Wa_h.T)
                cat_ps = psCAT.tile([128, 128], F32, space="PSUM", tag="catps")
                for bp in range(BC):
                    lhsT = _ap(awm, bp, [[BC * BC, S], [BC, BC]])
                    for q in range(4):
                        nc.tensor.matmul(
                            cat_ps[32 * q: 32 * q + BC, 0:128], lhsT,
                            eoWs[0:S, bp * H + q * 128: bp * H + q * 128 + 128],
                            start=(bp == 0), stop=False,
                            tile_position=(0, 32 * q), skip_group_check=True)
                for hc in range(4):
                    for q in range(4):
                        nc.tensor.matmul(
                            cat_ps[32 * q: 32 * q + BC, 0:128],
                            hT[:, hc * BC:(hc + 1) * BC],
                            WahK[:, hc * H + q * 128: hc * H + q * 128 + 128],
                            start=False, stop=(hc == 3),
                            tile_position=(0, 32 * q), skip_group_check=True)
                co = wk.tile([128, 128], F32, tag="co")
                for q in range(4):
                    nc.scalar.activation(co[32 * q: 32 * q + BC, 0:128],
                                         cat_ps[32 * q: 32 * q + BC, 0:128], AF.Tanh)

                # ============ AllGather #1: concat_out ============
                for q in range(4):
                    nc.sync.dma_start(
                        _ap(cc1_in, q * 128, [[H, BC], [1, 128]]),
                        co[32 * q: 32 * q + BC, 0:128])
                nc.gpsimd.collective_compute(
                    "AllGather", OP.bypass, replica_groups=rgroups,
                    ins=[cc1_in.ap().opt()], outs=[cc1_out.ap().opt()])
                coall = wk.tile([B, H], F32, tag="coall")
                nc.sync.dma_start(coall[:, :], cc1_out.ap())

                # concatT: [64,512] -> [128, 4*64]
                cT_ps = psS.tile([128, 4 * 64], F32, space="PSUM", tag="s")
                for hc in range(4):
                    nc.tensor.transpose(cT_ps[:, hc * 64:(hc + 1) * 64],
                                        coall[:, hc * 128:(hc + 1) * 128], ID64)
                cT = wk.tile([128, 4 * 64], F32, tag="cT")
                nc.vector.tensor_copy(cT[:, :], cT_ps[:, :])

                # ============ fc + per-bank stats ============
                staged = stg.tile([128, STG_W], F32, tag="staged")
                bmax8 = st.tile([128, NBANK * 8], F32, tag="bmax8")
                bidx8 = st.tile([128, NBANK * 8], U32, tag="bidx8")
                sume_b = st.tile([128, NBANK], F32, tag="sume_b")
                nc.vector.memset(bmax8[64:128, 48:56], -BIG)
                nc.vector.memset(bidx8[64:128, 48:56], 0)
                nc.vector.memset(sume_b[64:128, 6:7], 0.0)

                for bank in range(NBANK):
                    fc_ps = psFC.tile([128, 512], F32, space="PSUM", tag="fcps")
                    for kc in range(4):
                        for hf in range(2):
                            tt_i = 2 * bank + hf
                            if tt_i >= NTILE:
                                continue
                            w = 512 if tt_i < 12 else TAIL_W
                            pr = fc_ps[64 * hf:64 * hf + 64, 0:w]
                            nc.tensor.matmul(
                                pr,
                                cT[:, kc * 64:(kc + 1) * 64],
                                fcT[:, kc * VC + tt_i * 512: kc * VC + tt_i * 512 + w],
                                start=(kc == 0), stop=(kc == 3),
                                tile_position=(0, 64) if hf else None,
                                skip_group_check=True)
                    pmax = 128 if bank < 6 else 64
                    w_eff = 512 if bank < 6 else TAIL_W
                    s_sl = staged[0:pmax, bank * 512: bank * 512 + w_eff]
                    nc.scalar.copy(s_sl, fc_ps[0:pmax, 0:w_eff])
                    # exp in place on the psum bank (raw values already staged)
                    nc.scalar.activation(fc_ps[0:pmax, 0:w_eff], fc_ps[0:pmax, 0:w_eff],
                                         AF.Exp, accum_out=sume_b[0:pmax, bank:bank + 1])
                    nc.vector.max(out=bmax8[0:pmax, bank * 8:(bank + 1) * 8], in_=s_sl)
                    nc.vector.max_index(out=bidx8[0:pmax, bank * 8:(bank + 1) * 8],
                                        in_max=bmax8[0:pmax, bank * 8:(bank + 1) * 8],
                                        in_values=s_sl)

                # next step's gh matmuls: only need hT, so they fill the
                # PE pipe during the stats/AG2/token window
                if t < t_steps - 1:
                    gh_tiles = emit_gh(hT)

                # ============ local stats combine ============
                stats = st.tile([128, 4], F32, tag="stats")
                nc.vector.reduce_max(stats[:, 0:1], bmax8[:, :], axis=AX.X)
                nc.vector.reduce_sum(stats[:, 1:2], sume_b[:, :], axis=AX.X)
                cidx = st.tile([128, NBANK * 8], F32, tag="cidx")
                nc.vector.tensor_copy(cidx[:, :], bidx8[:, :])
                nc.vector.tensor_tensor(out=cidx[:, :], in0=cidx[:, :], in1=bankofs[:, :], op=OP.add)
                eqm = st.tile([128, NBANK * 8], F32, tag="eqm")
                nc.vector.tensor_tensor(out=eqm[:, :], in0=bmax8[:, :],
                                        in1=_ap(stats, 0, [[4, 128], [0, NBANK * 8]]),
                                        op=OP.is_equal)
                nc.vector.tensor_scalar(out=eqm[:, :], in0=eqm[:, :], scalar1=-BIG,
                                        scalar2=BIG, op0=OP.mult, op1=OP.add)
                nc.vector.tensor_tensor(out=cidx[:, :], in0=cidx[:, :], in1=eqm[:, :], op=OP.add)
                nc.vector.tensor_reduce(stats[:, 2:3], cidx[:, :], axis=AX.X, op=OP.min)
                nc.vector.memset(stats[:, 3:4], 0.0)

                # ============ AllGather #2: stats ============
                nc.sync.dma_start(cc2_in.ap(), stats[:, :])
                nc.gpsimd.collective_compute(
                    "AllGather", OP.bypass, replica_groups=rgroups,
                    ins=[cc2_in.ap().opt()], outs=[cc2_out.ap().opt()])
                gath = st.tile([128, 16 * 4], F32, tag="gath")
                gsrc = _ap(cc2_out, 0, [[4, 64], [512, NC], [256, 2], [1, 4]])
                nc.sync.dma_start(gath[0:64, :], gsrc)

                # global combine: token path first, on partitions 0:64 only
                lmaxs64 = _ap(gath, 0, [[64, 64], [4, 16]])
                gmax = st.tile([128, 1], F32, tag="gmax")
                nc.vector.reduce_max(gmax[0:64, :], lmaxs64, axis=AX.X)

                if t < t_steps - 1:
                    eq2 = st.tile([64, 16], F32, tag="eq2")
                    nc.vector.tensor_tensor(out=eq2[:, :], in0=lmaxs64,
                                            in1=_ap(gmax, 0, [[1, 64], [0, 16]]),
                                            op=OP.is_equal)
                    nc.vector.tensor_scalar(out=eq2[:, :], in0=eq2[:, :], scalar1=-BIG,
                                            scalar2=BIG, op0=OP.mult, op1=OP.add)
                    nc.vector.tensor_tensor(out=eq2[:, :], in0=eq2[:, :],
                                            in1=_ap(gath, 2, [[64, 64], [4, 16]]), op=OP.add)
                    tokf = st.tile([64, 1], F32, tag="tokf")
                    nc.vector.tensor_reduce(tokf[:, :], eq2[:, :], axis=AX.X, op=OP.min)
                    nc.vector.tensor_copy(toks[:, t + 1:t + 2], tokf[0:B, :])

                    # my 8 tokens -> indices -> embedding gather -> xT
                    tok_ps = psS.tile([BC, 1], F32, space="PSUM", tag="s")
                    nc.tensor.matmul(tok_ps[:, :], sel[:, :], tokf[:, :],
                                     start=True, stop=True)
                    tok_i = wk.tile([BC, 1], I32, tag="toki")
                    nc.vector.tensor_copy(tok_i[:, :], tok_ps[:, :])
                    x_g = wk.tile([BC, E], F32, tag="xg")
                    nc.gpsimd.indirect_dma_start(
                        out=x_g[:, :], out_offset=None, in_=emb_d.ap(),
                        in_offset=bass.IndirectOffsetOnAxis(ap=tok_i[:, 0:1], axis=0))
                    xt_ps = psS.tile([128, 2 * BC], F32, space="PSUM", tag="s")
                    for ec in range(2):
                        nc.tensor.transpose(xt_ps[:, ec * BC:(ec + 1) * BC],
                                            x_g[:, ec * 128:(ec + 1) * 128], ID8)
                    xT_new = stp.tile([128, 2 * BC], F32, tag="xT")
                    nc.vector.tensor_copy(xT_new[:, :], xt_ps[:, :])
                    xT = xT_new

                # per-shard sumexps were computed without max subtraction
                # (logits are O(1)), so lse = ln(sum_j sume_j) directly
                nc.sync.dma_start(gath[64:128, :], gsrc)
                ssum = st.tile([128, 1], F32, tag="ssum")
                nc.vector.reduce_sum(ssum[:, :], _ap(gath, 1, [[64, 128], [4, 16]]),
                                     axis=AX.X)

                # ln(ssum) via exponent bits + quadratic + 2 Newton steps (Exp only)
                si = ssum[:, :].bitcast(I32)
                e_i = st.tile([128, 1], I32, tag="e_i")
                nc.vector.tensor_scalar(out=e_i[:, :], in0=si, scalar1=23,
                                        scalar2=None, op0=OP.arith_shift_right)
                e_f = st.tile([128, 1], F32, tag="e_f")
                nc.vector.tensor_copy(e_f[:, :], e_i[:, :])
                nc.vector.tensor_scalar(out=e_f[:, :], in0=e_f[:, :], scalar1=-127.0,
                                        scalar2=None, op0=OP.add)
                m_i = st.tile([128, 1], I32, tag="m_i")
                nc.vector.tensor_scalar(out=m_i[:, :], in0=si, scalar1=0x7FFFFF,
                                        scalar2=None, op0=OP.bitwise_and)
                nc.vector.tensor_scalar(out=m_i[:, :], in0=m_i[:, :], scalar1=0x3F800000,
                                        scalar2=None, op0=OP.bitwise_or)
                m_f = m_i[:, :].bitcast(F32)
                poly = st.tile([128, 1], F32, tag="poly")
                nc.vector.tensor_scalar(out=poly[:, :], in0=m_f, scalar1=LC2,
                                        scalar2=LC1, op0=OP.mult, op1=OP.add)
                nc.vector.tensor_tensor(out=poly[:, :], in0=poly[:, :], in1=m_f, op=OP.mult)
                nc.vector.tensor_scalar(out=poly[:, :], in0=poly[:, :], scalar1=LC0,
                                        scalar2=None, op0=OP.add)
                lnv = st.tile([128, 1], F32, tag="lnv")
                nc.vector.tensor_tensor(out=lnv[:, :], in0=poly[:, :], in1=e_f[:, :], op=OP.add)
                nc.vector.tensor_scalar(out=lnv[:, :], in0=lnv[:, :], scalar1=LN2,
                                        scalar2=None, op0=OP.mult)
                for _ in range(2):
                    nx = st.tile([128, 1], F32, tag="nx")
                    nc.scalar.activation(nx[:, :], lnv[:, :], AF.Exp, scale=-1.0)
                    nc.vector.tensor_tensor(out=nx[:, :], in0=nx[:, :], in1=ssum[:, :], op=OP.mult)
                    nc.vector.tensor_scalar(out=nx[:, :], in0=nx[:, :], scalar1=-1.0,
                                            scalar2=None, op0=OP.add)
                    nc.vector.tensor_tensor(out=lnv[:, :], in0=lnv[:, :], in1=nx[:, :], op=OP.add)

                nlse = st.tile([128, 1], F32, tag="nlse")
                nc.vector.tensor_scalar_mul(nlse[:, :], lnv[:, :], -1.0)

                # subtract lse in place, then write out
                nc.vector.tensor_scalar(out=staged[:, 0:3072], in0=staged[:, 0:3072],
                                        scalar1=nlse[:, :], scalar2=None, op0=OP.add)
                nc.vector.tensor_scalar(out=staged[0:64, 3072:STG_W],
                                        in0=staged[0:64, 3072:STG_W],
                                        scalar1=nlse[0:64, :], scalar2=None, op0=OP.add)
                dst0 = _ap(olog_d, t * VC, [[t_steps * VC, B], [1024, 6], [1, 512]])
                nc.sync.dma_start(dst0, staged[0:64, 0:3072].rearrange("p (a b) -> p a b", a=6))
                dst1 = _ap(olog_d, t * VC + 512, [[t_steps * VC, B], [1024, 6], [1, 512]])
                nc.sync.dma_start(dst1, staged[64:128, 0:3072].rearrange("p (a b) -> p a b", a=6))
                dst2 = _ap(olog_d, t * VC + 6144, [[t_steps * VC, B], [1, TAIL_W]])
                nc.sync.dma_start(dst2, staged[0:64, 3072:3072 + TAIL_W])

            # epilogue
            nc.sync.dma_start(oh_d.ap(), h_b[:, :])
            nc.vector.memset(toks[:, 0:1], float(SOS))
            nc.sync.dma_start(otok_d.ap(), toks[:, :])

    nc.compile()
    return nc


# ======================= host side =======================

_CACHE = {}


def _prep_inputs(inputs, t_steps=T):
    emb = np.ascontiguousarray(np.asarray(inputs["embedding"], dtype=np.float32))
    eh = np.asarray(inputs["encoder_hidden"], dtype=np.float32)
    eo = np.asarray(inputs["encoder_outputs"], dtype=np.float32)
    w_ih = np.asarray(inputs["w_ih"], dtype=np.float32)
    w_hh = np.asarray(inputs["w_hh"], dtype=np.float32)
    attn_w = np.asarray(inputs["attn_w"], dtype=np.float32)
    Wa = np.asarray(inputs["Wa"], dtype=np.float32)
    fc_w = np.asarray(inputs["fc_w"], dtype=np.float32)

    h0 = eh[0]                                   # [B, H]
    x0 = emb[SOS]                                # [E]
    x0T = np.zeros((128, 2 * BC), np.float32)
    for ec in range(2):
        x0T[:, ec * BC:(ec + 1) * BC] = np.repeat(
            x0[ec * 128:(ec + 1) * 128][:, None], BC, 1)
    wihK = w_ih.T.reshape(2, 128, 3 * H).transpose(1, 0, 2).reshape(128, 2 * 3 * H)
    whhK = w_hh.T.reshape(4, 128, 3 * H).transpose(1, 0, 2).reshape(128, 4 * 3 * H)
    Wah = Wa[:, 0:H]                             # [512g, 512h]
    WahK = Wah.T.reshape(4, 128, H).transpose(1, 0, 2).reshape(128, 4 * H)
    Wac = Wa[:, H:2 * H]
    eye8 = np.tile(np.eye(BC, dtype=np.float32).reshape(1, BC * BC), (128, 1))
    base_b = np.zeros((128, NBANK * 8), np.float32)
    for bank in range(NBANK):
        base_b[:, bank * 8:(bank + 1) * 8] = bank * 1024
    base_b[64:128, :] += 512

    in_maps = []
    for c in range(NC):
        bs = slice(c * BC, (c + 1) * BC)
        eo_c = eo[bs]                            # [BC, S, H]
        h0b = np.ascontiguousarray(h0[bs])
        h0T = h0b.T.reshape(4, 128, BC).transpose(1, 0, 2).reshape(128, 4 * BC)
        # epT[p, (gc*8+b)*64+s] = ep_c[b, s, gc*128+p], ep = eo @ attn_w.T
        ep_c = np.einsum("bsh,gh->bsg", eo_c, attn_w).astype(np.float32)
        epT = ep_c.transpose(2, 0, 1).reshape(4, 128, BC, S).transpose(
            1, 0, 2, 3).reshape(128, 4 * BC * S)
        eoW = np.einsum("bsh,gh->bsg", eo_c, Wac).astype(np.float32)
        eoWs = eoW.transpose(1, 0, 2).reshape(S, BC * H)
        fc_c = fc_w[c * VC:(c + 1) * VC]         # [VC, H]
        fcT = fc_c.T.reshape(4, 128, VC).transpose(1, 0, 2).reshape(128, 4 * VC)
        sel_m = np.zeros((B, BC), np.float32)
        for j in range(BC):
            sel_m[c * BC + j, j] = 1.0
        bofs = base_b + c * VC
        in_maps.append({
            "emb": emb, "x0T": x0T, "h0b": h0b, "h0T": h0T,
            "wihK": np.ascontiguousarray(wihK), "whhK": np.ascontiguousarray(whhK),
            "WahK": np.ascontiguousarray(WahK),
            "epT": np.ascontiguousarray(epT), "eoWs": np.ascontiguousarray(eoWs),
            "fcT": np.ascontiguousarray(fcT), "sel": sel_m,
            "eye8r": eye8, "bankofs": bofs.astype(np.float32),
        })
    return in_maps


def kernel(**inputs):
    if "nc" not in _CACHE:
        _CACHE["nc"] = build(T)
    nc = _CACHE["nc"]
    in_maps = _prep_inputs(inputs, T)
    res = bass_utils.run_bass_kernel_spmd(nc, in_maps, core_ids=list(range(NC)))
    outs = res.results
    dec = np.concatenate([outs[c]["o_logits"] for c in range(NC)], axis=2)
    h_fin = np.concatenate([outs[c]["o_h"] for c in range(NC)], axis=0)[None]
    return dec, h_fin
